# revision 12
# baseline (speedup 1.0000x reference)
"""Trainium2 Bass kernel for a pre-norm transformer block (attention + MLP).

Sharding: pure data-parallel over 8 cores. Core c handles batch b=c//2 and
query-row half rh=c%2 (512 tokens). K/V are computed for the full 1024-token
batch on every core (duplicated across the pair) so no collectives are needed.

Device layout: activations are feature-major (features on partitions, tokens
on the free dim) so matmul chains need no transposes. Host permutes tokens so
each core's own 512 query tokens are always columns 0:512. LayerNorm is done
feature-major with ones-matmul partition reductions (float32r full-rate
matmuls); softmax uses exp(sim) * mask01 (no max subtraction, exact because
masked logits contribute exp=0) with per-query sums obtained for free from a
ones-column appended to V.
"""

import numpy as np
import ml_dtypes
from contextlib import ExitStack

import concourse.bass as bass
from concourse.bacc import Bacc
import concourse.tile as tile
from concourse import mybir
from concourse.bass_utils import run_bass_kernel_spmd

F32 = mybir.dt.float32
F32R = mybir.dt.float32r
BF16 = mybir.dt.bfloat16
AF = mybir.ActivationFunctionType
ALU = mybir.AluOpType
BFNP = ml_dtypes.bfloat16

B, N, C = 4, 1024, 1024
H, D = 16, 64
DFF = 4096
R = 512          # own query rows per core
P = 128
KC = C // P      # 8 feature k-tiles
NT = N // P      # 8 token tiles
EPS = 1e-6

_CACHE: dict = {}


def _r32(ap):
    return ap.bitcast(F32R)


def _ln_fm(nc, ln_ps, ln_bc, vecp, sqp, tmpp, ones_kr, ones_c1, eps_sb,
           get_x, put_xn, n_chunks, tagpfx):
    """Streaming feature-major layernorm over KC partition tiles.

    get_x(k, chunk, use) -> sbuf fp32 AP [128, 512] for feature tile k,
    token chunk `chunk` (use is 0 for the stats pass, 1 for normalize).
    put_xn(k, chunk, t1_ap, ps_rstd_ap) stores (x-mu)*rstd.
    """
    inv_c = 1.0 / C
    for chunk in range(n_chunks):
        ps_s = ln_ps.tile([1, 512], F32, tag="lnstat", name=f"{tagpfx}s{chunk}")
        ps_q = ln_ps.tile([1, 512], F32, tag="lnstat", name=f"{tagpfx}q{chunk}")
        for k in range(KC):
            xc = get_x(k, chunk, 0)
            xcr = sqp.tile([P, 512], F32R, tag="xcr", name=f"{tagpfx}xr{chunk}_{k}", bufs=2)
            nc.vector.tensor_copy(out=xcr[:], in_=xc)
            sqc = sqp.tile([P, 512], F32R, tag="sq", name=f"{tagpfx}sq{chunk}_{k}", bufs=2)
            nc.vector.tensor_mul(sqc[:], xc, xc)
            nc.tensor.matmul(ps_s[:], ones_kr[:], xcr[:],
                             start=(k == 0), stop=(k == KC - 1))
            nc.tensor.matmul(ps_q[:], ones_kr[:], sqc[:],
                             start=(k == 0), stop=(k == KC - 1))
        mu = vecp.tile([1, 512], F32, tag="vec", name=f"{tagpfx}mu{chunk}")
        var = vecp.tile([1, 512], F32, tag="vec", name=f"{tagpfx}var{chunk}")
        rstd = vecp.tile([1, 512], F32, tag="vec", name=f"{tagpfx}rstd{chunk}")
        nc.scalar.mul(mu[:], ps_s[:], inv_c)
        nc.scalar.mul(var[:], ps_q[:], inv_c)          # E[x^2]
        msq = vecp.tile([1, 512], F32, tag="vec", name=f"{tagpfx}msq{chunk}")
        nc.vector.tensor_mul(msq[:], mu[:], mu[:])
        nc.vector.tensor_sub(var[:], var[:], msq[:])
        nc.scalar.activation(var[:], var[:], AF.Sqrt, bias=eps_sb[:])
        nc.vector.reciprocal(rstd[:], var[:])
        mu_r = vecp.tile([1, 512], F32R, tag="vecr", name=f"{tagpfx}mur{chunk}", bufs=2)
        rstd_r = vecp.tile([1, 512], F32R, tag="vecr", name=f"{tagpfx}rsr{chunk}", bufs=2)
        nc.scalar.copy(mu_r[:], mu[:])
        nc.scalar.copy(rstd_r[:], rstd[:])
        ps_mu = ln_bc.tile([P, 512], F32, tag="lnbc", name=f"{tagpfx}bmu{chunk}")
        ps_rstd = ln_bc.tile([P, 512], F32, tag="lnbc", name=f"{tagpfx}brs{chunk}")
        nc.tensor.matmul(ps_mu[:], ones_c1[:, 0:P], mu_r[:],
                         start=True, stop=True)
        nc.tensor.matmul(ps_rstd[:], ones_c1[:, 0:P], rstd_r[:],
                         start=True, stop=True)
        for k in range(KC):
            xc = get_x(k, chunk, 1)
            t1 = tmpp.tile([P, 512], F32, tag="tmp", name=f"{tagpfx}t{chunk}_{k}")
            nc.vector.tensor_sub(t1[:], xc, ps_mu[:])
            put_xn(k, chunk, t1[:], ps_rstd[:])


def _build():
    nc = Bacc()
    io = {}
    io["xT"] = nc.dram_tensor("xT", [C, N], F32, kind="ExternalInput")
    io["mskT"] = nc.dram_tensor("mskT", [N, R], BF16, kind="ExternalInput")
    for nm, shp in [("wq", [C, C]), ("wk", [C, C]), ("wv", [C, C]),
                    ("wo", [C, C]), ("w1", [C, DFF]), ("w2", [DFF, C])]:
        io[nm] = nc.dram_tensor(nm, shp, BF16, kind="ExternalInput")
    for nm, n_ in [("bq", C), ("bk", C), ("bv", C), ("bo", C), ("b1", DFF), ("b2", C)]:
        io[nm] = nc.dram_tensor(nm, [n_], F32, kind="ExternalInput")
    io["yT"] = nc.dram_tensor("yT", [C, R], F32, kind="ExternalOutput")

    def bias_cols(name, n_):
        # bias vector [n_] -> sbuf [128, n_//128], col m = b[m*128:(m+1)*128]
        return bass.AP(tensor=io[name][:].tensor, offset=0, ap=[[1, P], [P, n_ // P]])

    with tile.TileContext(nc) as tc, ExitStack() as ctx:
        # ---- long-lived sbuf pools (stack: first opened = last closed)
        const = ctx.enter_context(tc.tile_pool(name="const", bufs=1))
        x2p = ctx.enter_context(tc.tile_pool(name="x2p", bufs=KC))
        xn2p = ctx.enter_context(tc.tile_pool(name="xn2p", bufs=KC))
        yp = ctx.enter_context(tc.tile_pool(name="yp", bufs=2))
        mskp = ctx.enter_context(tc.tile_pool(name="mskp", bufs=NT))
        qtp = ctx.enter_context(tc.tile_pool(name="qtp", bufs=KC))
        ktp = ctx.enter_context(tc.tile_pool(name="ktp", bufs=KC))
        vtp = ctx.enter_context(tc.tile_pool(name="vtp", bufs=NT))
        otp = ctx.enter_context(tc.tile_pool(name="otp", bufs=KC))
        wrow = ctx.enter_context(tc.tile_pool(name="wrow", bufs=12))
        vecp = ctx.enter_context(tc.tile_pool(name="vecp", bufs=4))
        tmpp = ctx.enter_context(tc.tile_pool(name="tmpp", bufs=3))
        sqp = ctx.enter_context(tc.tile_pool(name="sqp", bufs=3))

        # ---- constants
        bq_sb = const.tile([P, C // P], F32)
        bk_sb = const.tile([P, C // P], F32)
        bo_sb = const.tile([P, C // P], F32)
        b1_sb = const.tile([P, DFF // P], F32)
        b2_sb = const.tile([P, C // P], F32)
        nc.sync.dma_start(out=bq_sb[:], in_=bias_cols("bq", C))
        nc.sync.dma_start(out=bk_sb[:], in_=bias_cols("bk", C))
        nc.sync.dma_start(out=bo_sb[:], in_=bias_cols("bo", C))
        nc.sync.dma_start(out=b1_sb[:], in_=bias_cols("b1", DFF))
        nc.sync.dma_start(out=b2_sb[:], in_=bias_cols("b2", C))
        bv_b = const.tile([P, C], F32)
        nc.sync.dma_start(out=bv_b[:], in_=bass.AP(tensor=io["bv"][:].tensor,
                                                   offset=0, ap=[[0, P], [1, C]]))
        ones_kf = const.tile([P, 1], F32)
        nc.vector.memset(ones_kf[:], 1.0)
        ones_kr = const.tile([P, 1], F32R)
        nc.vector.tensor_copy(out=ones_kr[:], in_=ones_kf[:])
        ones_cf = const.tile([1, P], F32)
        nc.vector.memset(ones_cf[:], 1.0)
        ones_c1 = const.tile([1, P], F32R)
        nc.vector.tensor_copy(out=ones_c1[:], in_=ones_cf[:])
        eps_sb = const.tile([1, 1], F32)
        nc.vector.memset(eps_sb[:], EPS)

        msk_sb = [mskp.tile([P, R], BF16, tag="msk", name=f"msk{i}") for i in range(NT)]
        for t in range(NT):
            nc.sync.dma_start(out=msk_sb[t][:], in_=io["mskT"][t * P:(t + 1) * P, :])

        # resident activation tiles
        qt = [qtp.tile([P, R], BF16, tag="qt", name=f"qt{i}") for i in range(KC)]
        kt = [ktp.tile([P, N], BF16, tag="kt", name=f"kt{i}") for i in range(KC)]
        vt = [vtp.tile([P, H * (D + 1)], BF16, tag="vt", name=f"vt{i}") for i in range(NT)]
        ot = [otp.tile([P, R], BF16, tag="ot", name=f"ot{i}") for i in range(KC)]
        x2 = [x2p.tile([P, R], F32, tag="x2", name=f"x2_{i}") for i in range(KC)]
        xn2 = [xn2p.tile([P, R], BF16, tag="xn2", name=f"xn2_{i}") for i in range(KC)]

        # ================= LN1 + Q/K/V projections =================
        with tc.tile_pool(name="xn1p", bufs=KC) as xn1p, \
             tc.tile_pool(name="xs1", bufs=3) as xs1, \
             tc.tile_pool(name="ln_ps", bufs=2, space="PSUM") as ln_ps, \
             tc.tile_pool(name="ln_bc", bufs=2, space="PSUM") as ln_bc, \
             tc.tile_pool(name="mm_ps", bufs=3, space="PSUM") as mm_ps:
            xn1 = [xn1p.tile([P, N], BF16, tag="xn1", name=f"xn1_{i}") for i in range(KC)]

            def get_x1(k, chunk, use):
                xc = xs1.tile([P, 512], F32, tag="xs", name=f"xa{use}_{chunk}_{k}")
                nc.gpsimd.dma_start(out=xc[:], in_=io["xT"][k * P:(k + 1) * P,
                                                            chunk * 512:(chunk + 1) * 512])
                return xc[:]

            def put_xn1(k, chunk, t1, ps_rstd):
                nc.vector.tensor_tensor(xn1[k][:, chunk * 512:(chunk + 1) * 512],
                                        t1, ps_rstd, op=ALU.mult)

            _ln_fm(nc, ln_ps, ln_bc, vecp, sqp, tmpp, ones_kr, ones_c1, eps_sb,
                   get_x1, put_xn1, 2, "ln1")

            # ---- Q projection (own tokens only)
            wq_sb = [wrow.tile([P, C], BF16, tag="w", name=f"wq{i}") for i in range(KC)]
            for k in range(KC):
                nc.sync.dma_start(out=wq_sb[k][:], in_=io["wq"][k * P:(k + 1) * P, :])
            for m in range(KC):
                ps = mm_ps.tile([P, 512], F32, tag="mm")
                for k in range(KC):
                    nc.tensor.matmul(ps[:], wq_sb[k][:, m * P:(m + 1) * P], xn1[k][:, 0:R],
                                     start=(k == 0), stop=(k == KC - 1))
                nc.scalar.activation(qt[m][:], ps[:], AF.Identity, bias=bq_sb[:, m:m + 1])

            # ---- K projection (all tokens)
            wk_sb = [wrow.tile([P, C], BF16, tag="w", name=f"wk{i}") for i in range(KC)]
            for k in range(KC):
                nc.sync.dma_start(out=wk_sb[k][:], in_=io["wk"][k * P:(k + 1) * P, :])
            for m in range(KC):
                for nn_ in range(2):
                    ps = mm_ps.tile([P, 512], F32, tag="mm")
                    for k in range(KC):
                        nc.tensor.matmul(ps[:], wk_sb[k][:, m * P:(m + 1) * P],
                                         xn1[k][:, nn_ * 512:(nn_ + 1) * 512],
                                         start=(k == 0), stop=(k == KC - 1))
                    nc.scalar.activation(kt[m][:, nn_ * 512:(nn_ + 1) * 512], ps[:],
                                         AF.Identity, bias=bk_sb[:, m:m + 1])

            # ---- V projection (token-major, ones column per head for row sums)
            wv_sb = [wrow.tile([P, C], BF16, tag="w", name=f"wv{i}") for i in range(KC)]
            for k in range(KC):
                nc.sync.dma_start(out=wv_sb[k][:], in_=io["wv"][k * P:(k + 1) * P, :])
            for t in range(NT):
                vre = vt[t][:].rearrange("p (h j) -> p h j", j=D + 1)
                nc.vector.memset(vre[:, :, D:D + 1], 1.0)
                for nn_ in range(2):
                    ps = mm_ps.tile([P, 512], F32, tag="mm")
                    for k in range(KC):
                        nc.tensor.matmul(ps[:], xn1[k][:, t * P:(t + 1) * P],
                                         wv_sb[k][:, nn_ * 512:(nn_ + 1) * 512],
                                         start=(k == 0), stop=(k == KC - 1))
                    dst = vre[:, nn_ * 8:(nn_ + 1) * 8, 0:D]
                    nc.vector.tensor_tensor(
                        dst, ps[:].rearrange("p (h j) -> p h j", j=D),
                        bv_b[:, nn_ * 512:(nn_ + 1) * 512].rearrange("p (h j) -> p h j", j=D),
                        op=ALU.add)

        # ================= attention (head-skewed pipeline) =================
        with tc.tile_pool(name="simps", bufs=3, space="PSUM") as simps, \
             tc.tile_pool(name="ops", bufs=2, space="PSUM") as ops_, \
             tc.tile_pool(name="rbps", bufs=2, space="PSUM") as rbps, \
             tc.tile_pool(name="ep", bufs=3) as ep, \
             tc.tile_pool(name="ap_", bufs=11) as ap_, \
             tc.tile_pool(name="rbp", bufs=2) as rbp:

            a_tiles = {}

            def emit_qk(h):
                kth = kt[h // 2][(h % 2) * D:(h % 2) * D + D, :]
                qth = qt[h // 2][(h % 2) * D:(h % 2) * D + D, :]
                for tk in range(NT):
                    ps_sim = simps.tile([P, R], F32, tag="sim", name=f"sim{h}_{tk}")
                    nc.tensor.matmul(ps_sim[:], kth[:, tk * P:(tk + 1) * P], qth[:],
                                     start=True, stop=True)
                    e = ep.tile([P, R], BF16, tag="e", name=f"e{h}_{tk}")
                    nc.scalar.activation(e[:], ps_sim[:], AF.Exp)
                    a = ap_.tile([P, R], BF16, tag="a", name=f"a{h}_{tk}")
                    nc.vector.tensor_mul(a[:], e[:], msk_sb[tk][:])
                    a_tiles[(h, tk)] = a

            def emit_o(h):
                ps_o = ops_.tile([D + 1, R], F32, tag="o", name=f"o{h}")
                for tk in range(NT):
                    vre = vt[tk][:].rearrange("p (h j) -> p h j", j=D + 1)
                    nc.tensor.matmul(ps_o[:], vre[:, h, 0:D + 1], a_tiles[(h, tk)][:],
                                     start=(tk == 0), stop=(tk == NT - 1))
                    del a_tiles[(h, tk)]
                rec = vecp.tile([1, R], F32, tag="vec", name=f"rec{h}")
                nc.vector.reciprocal(rec[:], ps_o[D:D + 1, :])
                rec_r = vecp.tile([1, R], F32R, tag="vecr", name=f"recr{h}", bufs=2)
                nc.scalar.copy(rec_r[:], rec[:])
                ps_rb = rbps.tile([D, R], F32, tag="rb", name=f"rb{h}")
                nc.tensor.matmul(ps_rb[:], ones_c1[:, 0:D], rec_r[:],
                                 start=True, stop=True)
                rb = rbp.tile([D, R], F32, tag="rbs", name=f"rbs{h}")
                nc.scalar.copy(rb[:], ps_rb[:])
                nc.vector.tensor_tensor(ot[h // 2][(h % 2) * D:(h % 2) * D + D, :],
                                        ps_o[0:D, :], rb[:], op=ALU.mult)

            emit_qk(0)
            for h in range(1, H):
                emit_qk(h)
                emit_o(h - 1)
            emit_o(H - 1)

        # ================= attn out projection + residual =================
        with tc.tile_pool(name="xres", bufs=3) as xresp, \
             tc.tile_pool(name="mm_ps2", bufs=3, space="PSUM") as mm_ps:
            wo_sb = [wrow.tile([P, C], BF16, tag="w", name=f"wo{i}") for i in range(KC)]
            for k in range(KC):
                nc.sync.dma_start(out=wo_sb[k][:], in_=io["wo"][k * P:(k + 1) * P, :])
            for m in range(KC):
                ps = mm_ps.tile([P, 512], F32, tag="mm")
                for k in range(KC):
                    nc.tensor.matmul(ps[:], wo_sb[k][:, m * P:(m + 1) * P], ot[k][:],
                                     start=(k == 0), stop=(k == KC - 1))
                t1 = tmpp.tile([P, 512], F32, tag="tmp")
                nc.scalar.activation(t1[:], ps[:], AF.Identity, bias=bo_sb[:, m:m + 1])
                xr = xresp.tile([P, R], F32, tag="xr")
                nc.sync.dma_start(out=xr[:], in_=io["xT"][m * P:(m + 1) * P, 0:R])
                nc.vector.tensor_add(x2[m][:], t1[:], xr[:])

        # ================= LN2 =================
        with tc.tile_pool(name="ln_ps2", bufs=2, space="PSUM") as ln_ps, \
             tc.tile_pool(name="ln_bc2", bufs=2, space="PSUM") as ln_bc:

            def get_x2(k, chunk, use):
                return x2[k][:]

            def put_xn2(k, chunk, t1, ps_rstd):
                nc.vector.tensor_tensor(xn2[k][:], t1, ps_rstd, op=ALU.mult)

            _ln_fm(nc, ln_ps, ln_bc, vecp, sqp, tmpp, ones_kr, ones_c1, eps_sb,
                   get_x2, put_xn2, 1, "ln2")

        # ================= MLP =================
        # fc1 + gelu: process w1 in 4 column groups of 1024
        h1p = ctx.enter_context(tc.tile_pool(name="h1p", bufs=DFF // P))
        h1 = [h1p.tile([P, R], BF16, tag="h1", name=f"h1_{i}") for i in range(DFF // P)]
        with tc.tile_pool(name="mm_ps3", bufs=3, space="PSUM") as mm_ps:
            for cg in range(4):
                w1_sb = [wrow.tile([P, C], BF16, tag="w", name=f"w1_{cg}_{i}")
                         for i in range(KC)]
                for k in range(KC):
                    nc.sync.dma_start(out=w1_sb[k][:],
                                      in_=io["w1"][k * P:(k + 1) * P, cg * C:(cg + 1) * C])
                for m in range(KC):
                    ps = mm_ps.tile([P, 512], F32, tag="mm")
                    for k in range(KC):
                        nc.tensor.matmul(ps[:], w1_sb[k][:, m * P:(m + 1) * P], xn2[k][:],
                                         start=(k == 0), stop=(k == KC - 1))
                    om = cg * KC + m
                    nc.scalar.activation(h1[om][:], ps[:], AF.Gelu_apprx_tanh,
                                         bias=b1_sb[:, om:om + 1])

        # fc2 + residual: two output column groups of 512 (4 psum banks each)
        with tc.tile_pool(name="fc2ps", bufs=4, space="PSUM") as fc2ps:
            for mg in range(2):
                ps_list = [fc2ps.tile([P, 512], F32, tag="fc2", name=f"fc2ps{mg}_{i}")
                           for i in range(4)]
                w2_sb = [wrow.tile([P, 512], BF16, tag="w2s", name=f"w2_{mg}_{i}", bufs=6)
                         for i in range(DFF // P)]
                for k in range(DFF // P):
                    nc.sync.dma_start(out=w2_sb[k][:],
                                      in_=io["w2"][k * P:(k + 1) * P, mg * 512:(mg + 1) * 512])
                for k in range(DFF // P):
                    for m in range(4):
                        nc.tensor.matmul(ps_list[m][:], w2_sb[k][:, m * P:(m + 1) * P],
                                         h1[k][:], start=(k == 0), stop=(k == DFF // P - 1))
                for m in range(4):
                    om = mg * 4 + m
                    t2 = tmpp.tile([P, 512], F32, tag="tmp", name=f"t2_{om}")
                    nc.scalar.activation(t2[:], ps_list[m][:], AF.Identity,
                                         bias=b2_sb[:, om:om + 1])
                    y_sb = yp.tile([P, R], F32, tag="y", name=f"y{om}")
                    nc.vector.tensor_add(y_sb[:], t2[:], x2[om][:])
                    nc.sync.dma_start(out=io["yT"][om * P:(om + 1) * P, :], in_=y_sb[:])

    if not nc.is_finalized():
        nc.finalize()
    return nc


def _get_nc():
    if "nc" not in _CACHE:
        _CACHE["nc"] = _build()
    return _CACHE["nc"]


def _prep_in_maps(inputs):
    x = np.asarray(inputs["x"], dtype=np.float32)
    mask = np.asarray(inputs["mask"])
    scale = float(D) ** -0.5
    wq = (np.asarray(inputs["wq"], np.float32) * scale).astype(BFNP)
    bq = (np.asarray(inputs["bq"], np.float32) * scale).astype(np.float32)
    wkv = np.asarray(inputs["wkv"], np.float32)
    bkv = np.asarray(inputs["bkv"], np.float32)
    wk = np.ascontiguousarray(wkv[:, :C]).astype(BFNP)
    wv = np.ascontiguousarray(wkv[:, C:]).astype(BFNP)
    bk = np.ascontiguousarray(bkv[:C]).astype(np.float32)
    bv = np.ascontiguousarray(bkv[C:]).astype(np.float32)
    wo = np.asarray(inputs["wo"], np.float32).astype(BFNP)
    bo = np.asarray(inputs["bo"], np.float32)
    w1 = np.asarray(inputs["w1"], np.float32).astype(BFNP)
    b1 = np.asarray(inputs["b1"], np.float32)
    w2 = np.asarray(inputs["w2"], np.float32).astype(BFNP)
    b2 = np.asarray(inputs["b2"], np.float32)
    mask01 = mask.astype(np.float32)

    shared = dict(wq=wq, wk=wk, wv=wv, wo=wo, w1=w1, w2=w2,
                  bq=bq, bk=bk, bv=bv, bo=bo, b1=b1, b2=b2)
    in_maps = []
    for c in range(8):
        b = c // 2
        rh = c % 2
        own = np.arange(rh * R, rh * R + R)
        oth = np.arange((1 - rh) * R, (1 - rh) * R + R)
        perm = np.concatenate([own, oth])
        xT = np.ascontiguousarray(x[b].T[:, perm])
        mskT = np.ascontiguousarray(mask01[np.ix_(own, perm)].T).astype(BFNP)
        m = dict(shared)
        m["xT"] = xT
        m["mskT"] = mskT
        in_maps.append(m)
    return in_maps


def _assemble(results):
    out = np.empty((B, N, C), dtype=np.float32)
    for c in range(8):
        b = c // 2
        rh = c % 2
        out[b, rh * R:(rh + 1) * R, :] = results[c]["yT"].T
    return out


def run(inputs, trace=False):
    nc = _get_nc()
    in_maps = _prep_in_maps(inputs)
    res = run_bass_kernel_spmd(nc, in_maps, core_ids=list(range(8)), trace=trace)
    return _assemble(res.results), res


def kernel(**inputs):
    out, _ = run(inputs, trace=False)
    return out


# revision 19
# speedup vs baseline: 1.2321x; 1.2321x over previous
"""Trainium2 Bass kernel for a pre-norm transformer block (attention + MLP).

Sharding: pure data-parallel over 8 cores. Core c handles batch b=c//2 and
query-row half rh=c%2 (512 tokens). K/V are computed for the full 1024-token
batch on every core (duplicated across the pair) so no collectives are needed.

Device layout: activations are feature-major (features on partitions, tokens
on the free dim) so matmul chains need no transposes. Host permutes tokens so
each core's own 512 query tokens are always columns 0:512. LayerNorm is done
feature-major with ones-matmul partition reductions (float32r full-rate
matmuls); softmax uses exp(sim) * mask01 (no max subtraction, exact because
masked logits contribute exp=0) with per-query sums obtained for free from a
ones-column appended to V.
"""

import numpy as np
import ml_dtypes
from contextlib import ExitStack

import concourse.bass as bass
from concourse.bacc import Bacc
import concourse.tile as tile
from concourse import mybir
from concourse.bass_utils import run_bass_kernel_spmd

F32 = mybir.dt.float32
F32R = mybir.dt.float32r
BF16 = mybir.dt.bfloat16
AF = mybir.ActivationFunctionType
ALU = mybir.AluOpType
BFNP = ml_dtypes.bfloat16

B, N, C = 4, 1024, 1024
H, D = 16, 64
DFF = 4096
R = 512          # own query rows per core
P = 128
KC = C // P      # 8 feature k-tiles
NT = N // P      # 8 token tiles
EPS = 1e-6

_CACHE: dict = {}


def _r32(ap):
    return ap.bitcast(F32R)


def _ln_fm(nc, ln_ps, ln_bc, vecp, sqp, tmpp, ones_kr, ones_c1, eps_sb,
           get_x, put_xn, n_chunks, tagpfx):
    """Streaming feature-major layernorm over KC partition tiles.

    get_x(k, chunk, use) -> sbuf fp32 AP [128, 512] for feature tile k,
    token chunk `chunk` (use is 0 for the stats pass, 1 for normalize).
    put_xn(k, chunk, t1_ap, ps_rstd_ap) stores (x-mu)*rstd.
    """
    inv_c = 1.0 / C
    for chunk in range(n_chunks):
        ps_s = ln_ps.tile([1, 512], F32, tag="lnstat", name=f"{tagpfx}s{chunk}")
        ps_q = ln_ps.tile([1, 512], F32, tag="lnstat", name=f"{tagpfx}q{chunk}")
        for k in range(KC):
            xr, xf = get_x(k, chunk, 0)
            sqc = sqp.tile([P, 512], F32R, tag="sq", name=f"{tagpfx}sq{chunk}_{k}", bufs=2)
            nc.vector.tensor_mul(sqc[:], xf, xf)
            nc.tensor.matmul(ps_s[:], ones_kr[:], xr,
                             start=(k == 0), stop=(k == KC - 1))
            nc.tensor.matmul(ps_q[:], ones_kr[:], sqc[:],
                             start=(k == 0), stop=(k == KC - 1))
        mu = vecp.tile([1, 512], F32, tag="vec", name=f"{tagpfx}mu{chunk}")
        var = vecp.tile([1, 512], F32, tag="vec", name=f"{tagpfx}var{chunk}")
        rstd = vecp.tile([1, 512], F32, tag="vec", name=f"{tagpfx}rstd{chunk}")
        nc.scalar.mul(mu[:], ps_s[:], inv_c)
        nc.scalar.mul(var[:], ps_q[:], inv_c)          # E[x^2]
        msq = vecp.tile([1, 512], F32, tag="vec", name=f"{tagpfx}msq{chunk}")
        nc.vector.tensor_mul(msq[:], mu[:], mu[:])
        nc.vector.tensor_sub(var[:], var[:], msq[:])
        nc.scalar.activation(var[:], var[:], AF.Sqrt, bias=eps_sb[:])
        nc.vector.reciprocal(rstd[:], var[:])
        mu_r = vecp.tile([1, 512], F32R, tag="vecr", name=f"{tagpfx}mur{chunk}", bufs=2)
        rstd_r = vecp.tile([1, 512], F32R, tag="vecr", name=f"{tagpfx}rsr{chunk}", bufs=2)
        nc.scalar.copy(mu_r[:], mu[:])
        nc.scalar.copy(rstd_r[:], rstd[:])
        ps_mu = ln_bc.tile([P, 512], F32, tag="lnbc", name=f"{tagpfx}bmu{chunk}")
        ps_rstd = ln_bc.tile([P, 512], F32, tag="lnbc", name=f"{tagpfx}brs{chunk}")
        nc.tensor.matmul(ps_mu[:], ones_c1[:, 0:P], mu_r[:],
                         start=True, stop=True)
        nc.tensor.matmul(ps_rstd[:], ones_c1[:, 0:P], rstd_r[:],
                         start=True, stop=True)
        for k in range(KC):
            _, xf = get_x(k, chunk, 1)
            t1 = tmpp.tile([P, 512], F32, tag="tmp", name=f"{tagpfx}t{chunk}_{k}")
            nc.vector.tensor_sub(t1[:], xf, ps_mu[:])
            put_xn(k, chunk, t1[:], ps_rstd[:])


def _build():
    nc = Bacc()
    io = {}
    io["xT"] = nc.dram_tensor("xT", [C, N], F32R, kind="ExternalInput")
    io["mskT"] = nc.dram_tensor("mskT", [N, R], BF16, kind="ExternalInput")
    for nm, shp in [("wq", [C, C]), ("wk", [C, C]), ("wv", [C, C]),
                    ("wo", [C, C]), ("w1", [C, DFF]), ("w2", [DFF, C])]:
        io[nm] = nc.dram_tensor(nm, shp, BF16, kind="ExternalInput")
    for nm, n_ in [("bq", C), ("bk", C), ("bv", C), ("bo", C), ("b1", DFF), ("b2", C)]:
        io[nm] = nc.dram_tensor(nm, [n_], F32, kind="ExternalInput")
    io["sel2"] = nc.dram_tensor("sel2", [2, P], F32R, kind="ExternalInput")
    io["yT"] = nc.dram_tensor("yT", [C, R], F32, kind="ExternalOutput")

    def bias_cols(name, n_):
        # bias vector [n_] -> sbuf [128, n_//128], col m = b[m*128:(m+1)*128]
        return bass.AP(tensor=io[name][:].tensor, offset=0, ap=[[1, P], [P, n_ // P]])

    with tile.TileContext(nc) as tc, ExitStack() as ctx:
        # ---- long-lived sbuf pools (stack: first opened = last closed)
        const = ctx.enter_context(tc.tile_pool(name="const", bufs=1))
        x2p = ctx.enter_context(tc.tile_pool(name="x2p", bufs=KC))
        xn2p = ctx.enter_context(tc.tile_pool(name="xn2p", bufs=KC))
        yp = ctx.enter_context(tc.tile_pool(name="yp", bufs=2))
        mskp = ctx.enter_context(tc.tile_pool(name="mskp", bufs=NT))
        qtp = ctx.enter_context(tc.tile_pool(name="qtp", bufs=KC))
        ktp = ctx.enter_context(tc.tile_pool(name="ktp", bufs=KC))
        vtp = ctx.enter_context(tc.tile_pool(name="vtp", bufs=NT))
        otp = ctx.enter_context(tc.tile_pool(name="otp", bufs=KC))
        wrow = ctx.enter_context(tc.tile_pool(name="wrow", bufs=12))
        vecp = ctx.enter_context(tc.tile_pool(name="vecp", bufs=4))
        tmpp = ctx.enter_context(tc.tile_pool(name="tmpp", bufs=3))
        sqp = ctx.enter_context(tc.tile_pool(name="sqp", bufs=3))

        # ---- constants
        bq_sb = const.tile([P, C // P], F32)
        bk_sb = const.tile([P, C // P], F32)
        bo_sb = const.tile([P, C // P], F32)
        b1_sb = const.tile([P, DFF // P], F32)
        b2_sb = const.tile([P, C // P], F32)
        nc.sync.dma_start(out=bq_sb[:], in_=bias_cols("bq", C))
        nc.sync.dma_start(out=bk_sb[:], in_=bias_cols("bk", C))
        nc.sync.dma_start(out=bo_sb[:], in_=bias_cols("bo", C))
        nc.sync.dma_start(out=b1_sb[:], in_=bias_cols("b1", DFF))
        nc.sync.dma_start(out=b2_sb[:], in_=bias_cols("b2", C))
        bv_b = const.tile([P, C], F32)
        nc.sync.dma_start(out=bv_b[:], in_=bass.AP(tensor=io["bv"][:].tensor,
                                                   offset=0, ap=[[0, P], [1, C]]))
        ones_kf = const.tile([P, 1], F32)
        nc.vector.memset(ones_kf[:], 1.0)
        ones_kr = const.tile([P, 1], F32R)
        nc.vector.tensor_copy(out=ones_kr[:], in_=ones_kf[:])
        ones_cf = const.tile([1, P], F32)
        nc.vector.memset(ones_cf[:], 1.0)
        ones_c1 = const.tile([1, P], F32R)
        nc.vector.tensor_copy(out=ones_c1[:], in_=ones_cf[:])
        eps_sb = const.tile([1, 1], F32)
        nc.vector.memset(eps_sb[:], EPS)
        sela = const.tile([1, P], F32R)
        selb = const.tile([1, P], F32R)
        nc.sync.dma_start(out=sela[:], in_=io["sel2"][0:1, :])
        nc.sync.dma_start(out=selb[:], in_=io["sel2"][1:2, :])

        msk_sb = [mskp.tile([P, R], BF16, tag="msk", name=f"msk{i}") for i in range(NT)]
        for t in range(NT):
            nc.sync.dma_start(out=msk_sb[t][:], in_=io["mskT"][t * P:(t + 1) * P, :])

        # resident activation tiles
        qt = [qtp.tile([P, R], BF16, tag="qt", name=f"qt{i}") for i in range(KC)]
        kt = [ktp.tile([P, N], BF16, tag="kt", name=f"kt{i}") for i in range(KC)]
        vt = [vtp.tile([P, H * (D + 1)], BF16, tag="vt", name=f"vt{i}") for i in range(NT)]
        ot = [otp.tile([P, R], BF16, tag="ot", name=f"ot{i}") for i in range(KC)]
        x2 = [x2p.tile([P, R], F32, tag="x2", name=f"x2_{i}") for i in range(KC)]
        xn2 = [xn2p.tile([P, R], BF16, tag="xn2", name=f"xn2_{i}") for i in range(KC)]

        # ================= LN1 + Q/K/V projections =================
        with tc.tile_pool(name="xn1p", bufs=KC) as xn1p, \
             tc.tile_pool(name="xs1", bufs=3) as xs1, \
             tc.tile_pool(name="ln_ps", bufs=2, space="PSUM") as ln_ps, \
             tc.tile_pool(name="ln_bc", bufs=2, space="PSUM") as ln_bc, \
             tc.tile_pool(name="mm_ps", bufs=3, space="PSUM") as mm_ps:
            xn1 = [xn1p.tile([P, N], BF16, tag="xn1", name=f"xn1_{i}") for i in range(KC)]

            def get_x1(k, chunk, use):
                xc = xs1.tile([P, 512], F32R, tag="xs", name=f"xa{use}_{chunk}_{k}")
                nc.gpsimd.dma_start(out=xc[:], in_=io["xT"][k * P:(k + 1) * P,
                                                            chunk * 512:(chunk + 1) * 512])
                return xc[:], xc[:].bitcast(F32)

            def put_xn1(k, chunk, t1, ps_rstd):
                nc.vector.tensor_tensor(xn1[k][:, chunk * 512:(chunk + 1) * 512],
                                        t1, ps_rstd, op=ALU.mult)

            _ln_fm(nc, ln_ps, ln_bc, vecp, sqp, tmpp, ones_kr, ones_c1, eps_sb,
                   get_x1, put_xn1, 2, "ln1")

            # ---- Q projection (own tokens only)
            wq_sb = [wrow.tile([P, C], BF16, tag="w", name=f"wq{i}") for i in range(KC)]
            for k in range(KC):
                nc.sync.dma_start(out=wq_sb[k][:], in_=io["wq"][k * P:(k + 1) * P, :])
            for m in range(KC):
                ps = mm_ps.tile([P, 512], F32, tag="mm")
                for k in range(KC):
                    nc.tensor.matmul(ps[:], wq_sb[k][:, m * P:(m + 1) * P], xn1[k][:, 0:R],
                                     start=(k == 0), stop=(k == KC - 1))
                nc.scalar.activation(qt[m][:], ps[:], AF.Identity, bias=bq_sb[:, m:m + 1])

            # ---- K projection (all tokens)
            wk_sb = [wrow.tile([P, C], BF16, tag="w", name=f"wk{i}") for i in range(KC)]
            for k in range(KC):
                nc.sync.dma_start(out=wk_sb[k][:], in_=io["wk"][k * P:(k + 1) * P, :])
            for m in range(KC):
                for nn_ in range(2):
                    ps = mm_ps.tile([P, 512], F32, tag="mm")
                    for k in range(KC):
                        nc.tensor.matmul(ps[:], wk_sb[k][:, m * P:(m + 1) * P],
                                         xn1[k][:, nn_ * 512:(nn_ + 1) * 512],
                                         start=(k == 0), stop=(k == KC - 1))
                    nc.scalar.activation(kt[m][:, nn_ * 512:(nn_ + 1) * 512], ps[:],
                                         AF.Identity, bias=bk_sb[:, m:m + 1])

            # ---- V projection (token-major, ones column per head for row sums)
            wv_sb = [wrow.tile([P, C], BF16, tag="w", name=f"wv{i}") for i in range(KC)]
            for k in range(KC):
                nc.sync.dma_start(out=wv_sb[k][:], in_=io["wv"][k * P:(k + 1) * P, :])
            for t in range(NT):
                vre = vt[t][:].rearrange("p (h j) -> p h j", j=D + 1)
                nc.vector.memset(vre[:, :, D:D + 1], 1.0)
                for nn_ in range(2):
                    ps = mm_ps.tile([P, 512], F32, tag="mm")
                    for k in range(KC):
                        nc.tensor.matmul(ps[:], xn1[k][:, t * P:(t + 1) * P],
                                         wv_sb[k][:, nn_ * 512:(nn_ + 1) * 512],
                                         start=(k == 0), stop=(k == KC - 1))
                    dst = vre[:, nn_ * 8:(nn_ + 1) * 8, 0:D]
                    nc.vector.tensor_tensor(
                        dst, ps[:].rearrange("p (h j) -> p h j", j=D),
                        bv_b[:, nn_ * 512:(nn_ + 1) * 512].rearrange("p (h j) -> p h j", j=D),
                        op=ALU.add)

        # ================= attention (head-pair pipeline) =================
        with tc.tile_pool(name="simps", bufs=2, space="PSUM") as simps, \
             tc.tile_pool(name="ops", bufs=2, space="PSUM") as ops_, \
             tc.tile_pool(name="rbps", bufs=2, space="PSUM") as rbps, \
             tc.tile_pool(name="a2p", bufs=11) as a2p, \
             tc.tile_pool(name="smr", bufs=2) as smr:

            sums0 = smr.tile([1, (H // 2) * R], F32R, bufs=1)
            sums1 = smr.tile([1, (H // 2) * R], F32R, bufs=1)
            a_tiles = {}

            def emit_qk(p):
                h0, h1 = 2 * p, 2 * p + 1
                kth0 = kt[p][0:D, :]
                kth1 = kt[p][D:2 * D, :]
                qth0 = qt[p][0:D, :]
                qth1 = qt[p][D:2 * D, :]
                for tk in range(NT):
                    ps2 = simps.tile([P, 2 * R], F32, tag="sim", name=f"sim{p}_{tk}")
                    nc.tensor.matmul(ps2[:, 0:R], kth0[:, tk * P:(tk + 1) * P], qth0[:],
                                     start=True, stop=True)
                    nc.tensor.matmul(ps2[:, R:2 * R], kth1[:, tk * P:(tk + 1) * P], qth1[:],
                                     start=True, stop=True)
                    a2 = a2p.tile([P, 2 * R], BF16, tag="a", name=f"a{p}_{tk}")
                    nc.scalar.activation(a2[:], ps2[:], AF.Exp)
                    nc.vector.tensor_mul(a2[:, 0:R], a2[:, 0:R], msk_sb[tk][:])
                    nc.vector.tensor_mul(a2[:, R:2 * R], a2[:, R:2 * R], msk_sb[tk][:])
                    a_tiles[(p, tk)] = a2

            def emit_o(p):
                for hh in range(2):
                    h = 2 * p + hh
                    ps_o = ops_.tile([D + 1, R], F32, tag="o", name=f"o{h}")
                    for tk in range(NT):
                        vre = vt[tk][:].rearrange("p (h j) -> p h j", j=D + 1)
                        nc.tensor.matmul(ps_o[:], vre[:, h, 0:D + 1],
                                         a_tiles[(p, tk)][:, hh * R:(hh + 1) * R],
                                         start=(tk == 0), stop=(tk == NT - 1))
                    dst = sums0 if hh == 0 else sums1
                    nc.scalar.copy(dst[0:1, p * R:(p + 1) * R], ps_o[D:D + 1, :])
                    nc.scalar.copy(ot[p][hh * D:(hh + 1) * D, :], ps_o[0:D, :])
                for tk in range(NT):
                    del a_tiles[(p, tk)]

            emit_qk(0)
            for p in range(1, H // 2):
                emit_qk(p)
                emit_o(p - 1)
            emit_o(H // 2 - 1)

            # softmax normalization: broadcast sums per pair, fast reciprocal
            for p in range(H // 2):
                ps_rb = rbps.tile([P, R], F32, tag="rb", name=f"rb{p}")
                nc.tensor.matmul(ps_rb[:], sela[:], sums0[0:1, p * R:(p + 1) * R],
                                 start=True, stop=False)
                nc.tensor.matmul(ps_rb[:], selb[:], sums1[0:1, p * R:(p + 1) * R],
                                 start=False, stop=True)
                rec_sb = smr.tile([P, R], F32, tag="rec", name=f"rec{p}", bufs=2)
                nc.vector.reciprocal_approx_fast(out=rec_sb[:], in_=ps_rb[:])
                nc.vector.tensor_tensor(ot[p][:], ot[p][:], rec_sb[:], op=ALU.mult)

        # ================= attn out projection + residual =================
        with tc.tile_pool(name="xres", bufs=3) as xresp, \
             tc.tile_pool(name="mm_ps2", bufs=3, space="PSUM") as mm_ps:
            wo_sb = [wrow.tile([P, C], BF16, tag="w", name=f"wo{i}") for i in range(KC)]
            for k in range(KC):
                nc.sync.dma_start(out=wo_sb[k][:], in_=io["wo"][k * P:(k + 1) * P, :])
            for m in range(KC):
                ps = mm_ps.tile([P, 512], F32, tag="mm")
                for k in range(KC):
                    nc.tensor.matmul(ps[:], wo_sb[k][:, m * P:(m + 1) * P], ot[k][:],
                                     start=(k == 0), stop=(k == KC - 1))
                t1 = tmpp.tile([P, 512], F32, tag="tmp")
                nc.scalar.activation(t1[:], ps[:], AF.Identity, bias=bo_sb[:, m:m + 1])
                xr = xresp.tile([P, R], F32R, tag="xr")
                nc.sync.dma_start(out=xr[:], in_=io["xT"][m * P:(m + 1) * P, 0:R])
                nc.vector.tensor_add(x2[m][:], t1[:], xr[:].bitcast(F32))

        # ================= LN2 =================
        with tc.tile_pool(name="ln_ps2", bufs=2, space="PSUM") as ln_ps, \
             tc.tile_pool(name="ln_bc2", bufs=2, space="PSUM") as ln_bc:

            def get_x2(k, chunk, use):
                if use == 0:
                    xcr = sqp.tile([P, 512], F32R, tag="xcr", name=f"x2r{k}", bufs=2)
                    nc.vector.tensor_copy(out=xcr[:], in_=x2[k][:])
                    return xcr[:], x2[k][:]
                return None, x2[k][:]

            def put_xn2(k, chunk, t1, ps_rstd):
                nc.vector.tensor_tensor(xn2[k][:], t1, ps_rstd, op=ALU.mult)

            _ln_fm(nc, ln_ps, ln_bc, vecp, sqp, tmpp, ones_kr, ones_c1, eps_sb,
                   get_x2, put_xn2, 1, "ln2")

        # ================= MLP =================
        # fc1 + gelu: process w1 in 4 column groups of 1024
        h1p = ctx.enter_context(tc.tile_pool(name="h1p", bufs=DFF // P))
        h1 = [h1p.tile([P, R], BF16, tag="h1", name=f"h1_{i}") for i in range(DFF // P)]
        with tc.tile_pool(name="mm_ps3", bufs=3, space="PSUM") as mm_ps:
            for cg in range(4):
                w1_sb = [wrow.tile([P, C], BF16, tag="w", name=f"w1_{cg}_{i}")
                         for i in range(KC)]
                for k in range(KC):
                    nc.sync.dma_start(out=w1_sb[k][:],
                                      in_=io["w1"][k * P:(k + 1) * P, cg * C:(cg + 1) * C])
                for m in range(KC):
                    ps = mm_ps.tile([P, 512], F32, tag="mm")
                    for k in range(KC):
                        nc.tensor.matmul(ps[:], w1_sb[k][:, m * P:(m + 1) * P], xn2[k][:],
                                         start=(k == 0), stop=(k == KC - 1))
                    om = cg * KC + m
                    nc.scalar.activation(h1[om][:], ps[:], AF.Gelu_apprx_tanh,
                                         bias=b1_sb[:, om:om + 1])

        # fc2 + residual: two output column groups of 512 (4 psum banks each)
        with tc.tile_pool(name="fc2ps", bufs=4, space="PSUM") as fc2ps:
            for mg in range(2):
                ps_list = [fc2ps.tile([P, 512], F32, tag="fc2", name=f"fc2ps{mg}_{i}")
                           for i in range(4)]
                w2_sb = [wrow.tile([P, 512], BF16, tag="w2s", name=f"w2_{mg}_{i}", bufs=6)
                         for i in range(DFF // P)]
                for k in range(DFF // P):
                    nc.sync.dma_start(out=w2_sb[k][:],
                                      in_=io["w2"][k * P:(k + 1) * P, mg * 512:(mg + 1) * 512])
                for k in range(DFF // P):
                    for m in range(4):
                        nc.tensor.matmul(ps_list[m][:], w2_sb[k][:, m * P:(m + 1) * P],
                                         h1[k][:], start=(k == 0), stop=(k == DFF // P - 1))
                for m in range(4):
                    om = mg * 4 + m
                    t2 = tmpp.tile([P, 512], F32, tag="tmp", name=f"t2_{om}")
                    nc.scalar.activation(t2[:], ps_list[m][:], AF.Identity,
                                         bias=b2_sb[:, om:om + 1])
                    y_sb = yp.tile([P, R], F32, tag="y", name=f"y{om}")
                    nc.vector.tensor_add(y_sb[:], t2[:], x2[om][:])
                    nc.sync.dma_start(out=io["yT"][om * P:(om + 1) * P, :], in_=y_sb[:])

    if not nc.is_finalized():
        nc.finalize()
    return nc


def _get_nc():
    if "nc" not in _CACHE:
        _CACHE["nc"] = _build()
    return _CACHE["nc"]


def _prep_in_maps(inputs):
    x = np.asarray(inputs["x"], dtype=np.float32)
    mask = np.asarray(inputs["mask"])
    scale = float(D) ** -0.5
    wq = (np.asarray(inputs["wq"], np.float32) * scale).astype(BFNP)
    bq = (np.asarray(inputs["bq"], np.float32) * scale).astype(np.float32)
    wkv = np.asarray(inputs["wkv"], np.float32)
    bkv = np.asarray(inputs["bkv"], np.float32)
    wk = np.ascontiguousarray(wkv[:, :C]).astype(BFNP)
    wv = np.ascontiguousarray(wkv[:, C:]).astype(BFNP)
    bk = np.ascontiguousarray(bkv[:C]).astype(np.float32)
    bv = np.ascontiguousarray(bkv[C:]).astype(np.float32)
    wo = np.asarray(inputs["wo"], np.float32).astype(BFNP)
    bo = np.asarray(inputs["bo"], np.float32)
    w1 = np.asarray(inputs["w1"], np.float32).astype(BFNP)
    b1 = np.asarray(inputs["b1"], np.float32)
    w2 = np.asarray(inputs["w2"], np.float32).astype(BFNP)
    b2 = np.asarray(inputs["b2"], np.float32)
    mask01 = mask.astype(np.float32)

    sel2 = np.zeros((2, P), dtype=np.float32)
    sel2[0, 0:D] = 1.0
    sel2[1, D:2 * D] = 1.0
    shared = dict(wq=wq, wk=wk, wv=wv, wo=wo, w1=w1, w2=w2,
                  bq=bq, bk=bk, bv=bv, bo=bo, b1=b1, b2=b2, sel2=sel2)
    in_maps = []
    for c in range(8):
        b = c // 2
        rh = c % 2
        own = np.arange(rh * R, rh * R + R)
        oth = np.arange((1 - rh) * R, (1 - rh) * R + R)
        perm = np.concatenate([own, oth])
        xT = np.ascontiguousarray(x[b].T[:, perm])
        mskT = np.ascontiguousarray(mask01[np.ix_(own, perm)].T).astype(BFNP)
        m = dict(shared)
        m["xT"] = xT
        m["mskT"] = mskT
        in_maps.append(m)
    return in_maps


def _assemble(results):
    out = np.empty((B, N, C), dtype=np.float32)
    for c in range(8):
        b = c // 2
        rh = c % 2
        out[b, rh * R:(rh + 1) * R, :] = results[c]["yT"].T
    return out


def run(inputs, trace=False):
    nc = _get_nc()
    in_maps = _prep_in_maps(inputs)
    res = run_bass_kernel_spmd(nc, in_maps, core_ids=list(range(8)), trace=trace)
    return _assemble(res.results), res


def kernel(**inputs):
    out, _ = run(inputs, trace=False)
    return out


# revision 24
# speedup vs baseline: 1.2791x; 1.0381x over previous
"""Trainium2 Bass kernel for a pre-norm transformer block (attention + MLP).

Sharding: pure data-parallel over 8 cores. Core c handles batch b=c//2 and
query-row half rh=c%2 (512 tokens). K/V are computed for the full 1024-token
batch on every core (duplicated across the pair) so no collectives are needed.

Device layout: activations are feature-major (features on partitions, tokens
on the free dim) so matmul chains need no transposes. Host permutes tokens so
each core's own 512 query tokens are always columns 0:512. LayerNorm is done
feature-major with ones-matmul partition reductions (float32r full-rate
matmuls); softmax uses exp(sim) * mask01 (no max subtraction, exact because
masked logits contribute exp=0) with per-query sums obtained for free from a
ones-column appended to V.
"""

import numpy as np
import ml_dtypes
from contextlib import ExitStack

import concourse.bass as bass
from concourse.bacc import Bacc
import concourse.tile as tile
from concourse import mybir
from concourse.bass_utils import run_bass_kernel_spmd

F32 = mybir.dt.float32
F32R = mybir.dt.float32r
BF16 = mybir.dt.bfloat16
AF = mybir.ActivationFunctionType
ALU = mybir.AluOpType
BFNP = ml_dtypes.bfloat16

B, N, C = 4, 1024, 1024
H, D = 16, 64
DFF = 4096
R = 512          # own query rows per core
P = 128
KC = C // P      # 8 feature k-tiles
NT = N // P      # 8 token tiles
EPS = 1e-6

_CACHE: dict = {}


def _r32(ap):
    return ap.bitcast(F32R)


def _ln_fm(nc, ln_ps, ln_bc, vecp, sqp, tmpp, ones_kr, ones_c1, eps_sb,
           get_x, put_xn, n_chunks, tagpfx):
    """Streaming feature-major layernorm over KC partition tiles.

    get_x(k, chunk, use) -> sbuf fp32 AP [128, 512] for feature tile k,
    token chunk `chunk` (use is 0 for the stats pass, 1 for normalize).
    put_xn(k, chunk, t1_ap, ps_rstd_ap) stores (x-mu)*rstd.
    """
    inv_c = 1.0 / C
    for chunk in range(n_chunks):
        ps_s = ln_ps.tile([1, 512], F32, tag="lnstat", name=f"{tagpfx}s{chunk}")
        ps_q = ln_ps.tile([1, 512], F32, tag="lnstat", name=f"{tagpfx}q{chunk}")
        for k in range(KC):
            xr, xf = get_x(k, chunk, 0)
            sqc = sqp.tile([P, 512], F32R, tag="sq", name=f"{tagpfx}sq{chunk}_{k}", bufs=2)
            nc.vector.tensor_mul(sqc[:], xf, xf)
            nc.tensor.matmul(ps_s[:], ones_kr[:], xr,
                             start=(k == 0), stop=(k == KC - 1))
            nc.tensor.matmul(ps_q[:], ones_kr[:], sqc[:],
                             start=(k == 0), stop=(k == KC - 1))
        mu = vecp.tile([1, 512], F32, tag="vec", name=f"{tagpfx}mu{chunk}")
        var = vecp.tile([1, 512], F32, tag="vec", name=f"{tagpfx}var{chunk}")
        rstd = vecp.tile([1, 512], F32, tag="vec", name=f"{tagpfx}rstd{chunk}")
        nc.scalar.mul(mu[:], ps_s[:], inv_c)
        nc.scalar.mul(var[:], ps_q[:], inv_c)          # E[x^2]
        msq = vecp.tile([1, 512], F32, tag="vec", name=f"{tagpfx}msq{chunk}")
        nc.vector.tensor_mul(msq[:], mu[:], mu[:])
        nc.vector.tensor_sub(var[:], var[:], msq[:])
        nc.scalar.activation(var[:], var[:], AF.Sqrt, bias=eps_sb[:])
        nc.vector.reciprocal_approx_fast(out=rstd[:], in_=var[:])
        mu_r = vecp.tile([1, 512], F32R, tag="vecr", name=f"{tagpfx}mur{chunk}", bufs=2)
        rstd_r = vecp.tile([1, 512], F32R, tag="vecr", name=f"{tagpfx}rsr{chunk}", bufs=2)
        nc.scalar.copy(mu_r[:], mu[:])
        nc.scalar.copy(rstd_r[:], rstd[:])
        ps_mu = ln_bc.tile([P, 512], F32, tag="lnbc", name=f"{tagpfx}bmu{chunk}")
        ps_rstd = ln_bc.tile([P, 512], F32, tag="lnbc", name=f"{tagpfx}brs{chunk}")
        nc.tensor.matmul(ps_mu[:], ones_c1[:, 0:P], mu_r[:],
                         start=True, stop=True)
        nc.tensor.matmul(ps_rstd[:], ones_c1[:, 0:P], rstd_r[:],
                         start=True, stop=True)
        for k in range(KC):
            _, xf = get_x(k, chunk, 1)
            t1 = tmpp.tile([P, 512], F32, tag="tmp", name=f"{tagpfx}t{chunk}_{k}")
            nc.vector.tensor_sub(t1[:], xf, ps_mu[:])
            put_xn(k, chunk, t1[:], ps_rstd[:])


def _build():
    nc = Bacc()
    io = {}
    io["xT"] = nc.dram_tensor("xT", [C, N], F32R, kind="ExternalInput")
    io["mskT"] = nc.dram_tensor("mskT", [N, R], BF16, kind="ExternalInput")
    for nm, shp in [("wq", [C, C]), ("wk", [C, C]), ("wv", [C, C]),
                    ("wo", [C, C]), ("w1", [C, DFF]), ("w2", [DFF, C])]:
        io[nm] = nc.dram_tensor(nm, shp, BF16, kind="ExternalInput")
    for nm, n_ in [("bq", C), ("bk", C), ("bv", C), ("bo", C), ("b1", DFF), ("b2", C)]:
        io[nm] = nc.dram_tensor(nm, [n_], F32, kind="ExternalInput")
    io["sel2"] = nc.dram_tensor("sel2", [2, P], F32R, kind="ExternalInput")
    io["yT"] = nc.dram_tensor("yT", [C, R], F32, kind="ExternalOutput")

    def bias_cols(name, n_):
        # bias vector [n_] -> sbuf [128, n_//128], col m = b[m*128:(m+1)*128]
        return bass.AP(tensor=io[name][:].tensor, offset=0, ap=[[1, P], [P, n_ // P]])

    with tile.TileContext(nc) as tc, ExitStack() as ctx:
        # ---- long-lived sbuf pools (stack: first opened = last closed)
        const = ctx.enter_context(tc.tile_pool(name="const", bufs=1))
        x2p = ctx.enter_context(tc.tile_pool(name="x2p", bufs=KC))
        xn2p = ctx.enter_context(tc.tile_pool(name="xn2p", bufs=KC))
        yp = ctx.enter_context(tc.tile_pool(name="yp", bufs=2))
        mskp = ctx.enter_context(tc.tile_pool(name="mskp", bufs=NT))
        qtp = ctx.enter_context(tc.tile_pool(name="qtp", bufs=KC))
        ktp = ctx.enter_context(tc.tile_pool(name="ktp", bufs=KC))
        vtp = ctx.enter_context(tc.tile_pool(name="vtp", bufs=NT))
        otp = ctx.enter_context(tc.tile_pool(name="otp", bufs=KC))
        wrow = ctx.enter_context(tc.tile_pool(name="wrow", bufs=10))
        vecp = ctx.enter_context(tc.tile_pool(name="vecp", bufs=4))
        tmpp = ctx.enter_context(tc.tile_pool(name="tmpp", bufs=3))
        sqp = ctx.enter_context(tc.tile_pool(name="sqp", bufs=3))

        # ---- constants
        bq_sb = const.tile([P, C // P], F32)
        bk_sb = const.tile([P, C // P], F32)
        bo_sb = const.tile([P, C // P], F32)
        b1_sb = const.tile([P, DFF // P], F32)
        b2_sb = const.tile([P, C // P], F32)
        nc.sync.dma_start(out=bq_sb[:], in_=bias_cols("bq", C))
        nc.sync.dma_start(out=bk_sb[:], in_=bias_cols("bk", C))
        nc.sync.dma_start(out=bo_sb[:], in_=bias_cols("bo", C))
        nc.sync.dma_start(out=b1_sb[:], in_=bias_cols("b1", DFF))
        nc.sync.dma_start(out=b2_sb[:], in_=bias_cols("b2", C))
        bv_b = const.tile([P, C], F32)
        nc.sync.dma_start(out=bv_b[:], in_=bass.AP(tensor=io["bv"][:].tensor,
                                                   offset=0, ap=[[0, P], [1, C]]))
        ones_kf = const.tile([P, 1], F32)
        nc.vector.memset(ones_kf[:], 1.0)
        ones_kr = const.tile([P, 1], F32R)
        nc.vector.tensor_copy(out=ones_kr[:], in_=ones_kf[:])
        ones_cf = const.tile([1, P], F32)
        nc.vector.memset(ones_cf[:], 1.0)
        ones_c1 = const.tile([1, P], F32R)
        nc.vector.tensor_copy(out=ones_c1[:], in_=ones_cf[:])
        eps_sb = const.tile([1, 1], F32)
        nc.vector.memset(eps_sb[:], EPS)
        sela = const.tile([1, P], F32R)
        selb = const.tile([1, P], F32R)
        nc.sync.dma_start(out=sela[:], in_=io["sel2"][0:1, :])
        nc.sync.dma_start(out=selb[:], in_=io["sel2"][1:2, :])

        msk_sb = [mskp.tile([P, R], BF16, tag="msk", name=f"msk{i}") for i in range(NT)]
        for t in range(NT):
            nc.sync.dma_start(out=msk_sb[t][:], in_=io["mskT"][t * P:(t + 1) * P, :])

        # resident activation tiles
        qt = [qtp.tile([P, R], BF16, tag="qt", name=f"qt{i}") for i in range(KC)]
        kt = [ktp.tile([P, N], BF16, tag="kt", name=f"kt{i}") for i in range(KC)]
        vt = [vtp.tile([P, H * (D + 1)], BF16, tag="vt", name=f"vt{i}") for i in range(NT)]
        ot = [otp.tile([P, R], BF16, tag="ot", name=f"ot{i}") for i in range(KC)]
        x2 = [x2p.tile([P, R], F32, tag="x2", name=f"x2_{i}") for i in range(KC)]
        xn2 = [xn2p.tile([P, R], BF16, tag="xn2", name=f"xn2_{i}") for i in range(KC)]

        # ================= LN1 / Q / V0 then interleaved K+attention =================
        with tc.tile_pool(name="xn1p", bufs=KC) as xn1p, \
             tc.tile_pool(name="xs1", bufs=3) as xs1:
            xn1 = [xn1p.tile([P, N], BF16, tag="xn1", name=f"xn1_{i}") for i in range(KC)]

            with tc.tile_pool(name="ln_ps", bufs=2, space="PSUM") as ln_ps, \
                 tc.tile_pool(name="ln_bc", bufs=2, space="PSUM") as ln_bc, \
                 tc.tile_pool(name="mm_ps", bufs=3, space="PSUM") as mm_ps:

                # HAM warmup: dummy back-to-back matmuls while LN1 does DMA/DVE work
                wup = mm_ps.tile([P, 512], F32, tag="wup", bufs=1)
                for i in range(24):
                    nc.tensor.matmul(wup[:], msk_sb[0][:, 0:P], msk_sb[0][:],
                                     start=(i == 0), stop=(i == 23))
                wup_sb = vecp.tile([1, 1], F32, tag="vec", name="wupsb")
                nc.scalar.copy(wup_sb[:], wup[0:1, 0:1])

                def get_x1(k, chunk, use):
                    xc = xs1.tile([P, 512], F32R, tag="xs", name=f"xa{use}_{chunk}_{k}")
                    nc.gpsimd.dma_start(out=xc[:], in_=io["xT"][k * P:(k + 1) * P,
                                                                chunk * 512:(chunk + 1) * 512])
                    return xc[:], xc[:].bitcast(F32)

                def put_xn1(k, chunk, t1, ps_rstd):
                    nc.vector.tensor_tensor(xn1[k][:, chunk * 512:(chunk + 1) * 512],
                                            t1, ps_rstd, op=ALU.mult)

                _ln_fm(nc, ln_ps, ln_bc, vecp, sqp, tmpp, ones_kr, ones_c1, eps_sb,
                       get_x1, put_xn1, 2, "ln1")

                # ---- Q projection (own tokens only)
                wq_sb = [wrow.tile([P, C], BF16, tag="w", name=f"wq{i}") for i in range(KC)]
                for k in range(KC):
                    nc.sync.dma_start(out=wq_sb[k][:], in_=io["wq"][k * P:(k + 1) * P, :])
                for m in range(KC):
                    ps = mm_ps.tile([P, 512], F32, tag="mm")
                    for k in range(KC):
                        nc.tensor.matmul(ps[:], wq_sb[k][:, m * P:(m + 1) * P], xn1[k][:, 0:R],
                                         start=(k == 0), stop=(k == KC - 1))
                    nc.scalar.activation(qt[m][:], ps[:], AF.Identity, bias=bq_sb[:, m:m + 1])

                # ---- V projection chunk 0 (heads 0-7), token-major with ones column
                wv_sb = [wrow.tile([P, C], BF16, tag="w", name=f"wv{i}") for i in range(KC)]
                for k in range(KC):
                    nc.sync.dma_start(out=wv_sb[k][:], in_=io["wv"][k * P:(k + 1) * P, :])
                for t in range(NT):
                    vre = vt[t][:].rearrange("p (h j) -> p h j", j=D + 1)
                    nc.vector.memset(vre[:, :, D:D + 1], 1.0)
                    ps = mm_ps.tile([P, 512], F32, tag="mm")
                    for k in range(KC):
                        nc.tensor.matmul(ps[:], xn1[k][:, t * P:(t + 1) * P],
                                         wv_sb[k][:, 0:512],
                                         start=(k == 0), stop=(k == KC - 1))
                    nc.vector.tensor_tensor(
                        vre[:, 0:8, 0:D], ps[:].rearrange("p (h j) -> p h j", j=D),
                        bv_b[:, 0:512].rearrange("p (h j) -> p h j", j=D),
                        op=ALU.add)

            # ---- interleaved: K projection + V chunk 1 + attention pair pipeline
            with tc.tile_pool(name="simps", bufs=2, space="PSUM") as simps, \
                 tc.tile_pool(name="ops", bufs=1, space="PSUM") as ops_, \
                 tc.tile_pool(name="mmb", bufs=2, space="PSUM") as mmb, \
                 tc.tile_pool(name="rbps", bufs=1, space="PSUM") as rbps, \
                 tc.tile_pool(name="a2p", bufs=10) as a2p, \
                 tc.tile_pool(name="recp", bufs=2) as recp, \
                 tc.tile_pool(name="smr", bufs=2) as smr:

                wk_sb = [wrow.tile([P, C], BF16, tag="w", name=f"wk{i}") for i in range(KC)]
                for k in range(KC):
                    nc.sync.dma_start(out=wk_sb[k][:], in_=io["wk"][k * P:(k + 1) * P, :])
                wvh_sb = [wrow.tile([P, 512], BF16, tag="w2s", name=f"wvh{i}", bufs=8)
                          for i in range(KC)]
                for k in range(KC):
                    nc.sync.dma_start(out=wvh_sb[k][:],
                                      in_=io["wv"][k * P:(k + 1) * P, 512:1024])

                a_tiles = {}
                sums = {}

                def emit_k(p):
                    for nn_ in range(2):
                        ps = mmb.tile([P, 512], F32, tag="mm", name=f"kp{p}_{nn_}")
                        for k in range(KC):
                            nc.tensor.matmul(ps[:], wk_sb[k][:, p * P:(p + 1) * P],
                                             xn1[k][:, nn_ * 512:(nn_ + 1) * 512],
                                             start=(k == 0), stop=(k == KC - 1))
                        nc.scalar.activation(kt[p][:, nn_ * 512:(nn_ + 1) * 512], ps[:],
                                             AF.Identity, bias=bk_sb[:, p:p + 1])

                def emit_v1(t):
                    vre = vt[t][:].rearrange("p (h j) -> p h j", j=D + 1)
                    ps = mmb.tile([P, 512], F32, tag="mm", name=f"v1_{t}")
                    for k in range(KC):
                        nc.tensor.matmul(ps[:], xn1[k][:, t * P:(t + 1) * P],
                                         wvh_sb[k][:],
                                         start=(k == 0), stop=(k == KC - 1))
                    nc.vector.tensor_tensor(
                        vre[:, 8:16, 0:D], ps[:].rearrange("p (h j) -> p h j", j=D),
                        bv_b[:, 512:1024].rearrange("p (h j) -> p h j", j=D),
                        op=ALU.add)

                def emit_qk(p):
                    kth0 = kt[p][0:D, :]
                    kth1 = kt[p][D:2 * D, :]
                    qth0 = qt[p][0:D, :]
                    qth1 = qt[p][D:2 * D, :]
                    for tk in range(NT):
                        ps2 = simps.tile([P, 2 * R], F32, tag="sim", name=f"sim{p}_{tk}")
                        nc.tensor.matmul(ps2[:, 0:R], kth0[:, tk * P:(tk + 1) * P], qth0[:],
                                         start=True, stop=True)
                        nc.tensor.matmul(ps2[:, R:2 * R], kth1[:, tk * P:(tk + 1) * P], qth1[:],
                                         start=True, stop=True)
                        a2 = a2p.tile([P, 2 * R], BF16, tag="a", name=f"a{p}_{tk}")
                        nc.scalar.activation(a2[:], ps2[:], AF.Exp)
                        nc.vector.tensor_mul(a2[:, 0:R], a2[:, 0:R], msk_sb[tk][:])
                        nc.vector.tensor_mul(a2[:, R:2 * R], a2[:, R:2 * R], msk_sb[tk][:])
                        a_tiles[(p, tk)] = a2

                def emit_o(p):
                    s0 = smr.tile([1, R], F32R, tag="s0", name=f"s0_{p}")
                    s1 = smr.tile([1, R], F32R, tag="s1", name=f"s1_{p}")
                    sums[p] = (s0, s1)
                    for hh in range(2):
                        h = 2 * p + hh
                        ps_o = ops_.tile([D + 1, R], F32, tag="o", name=f"o{h}")
                        for tk in range(NT):
                            vre = vt[tk][:].rearrange("p (h j) -> p h j", j=D + 1)
                            nc.tensor.matmul(ps_o[:], vre[:, h, 0:D + 1],
                                             a_tiles[(p, tk)][:, hh * R:(hh + 1) * R],
                                             start=(tk == 0), stop=(tk == NT - 1))
                        dst = s0 if hh == 0 else s1
                        nc.scalar.copy(dst[0:1, :], ps_o[D:D + 1, :])
                        nc.scalar.copy(ot[p][hh * D:(hh + 1) * D, :], ps_o[0:D, :])
                    for tk in range(NT):
                        del a_tiles[(p, tk)]

                def emit_norm(p):
                    s0, s1 = sums.pop(p)
                    ps_rb = rbps.tile([P, R], F32, tag="rb", name=f"rb{p}")
                    nc.tensor.matmul(ps_rb[:], sela[:], s0[0:1, :],
                                     start=True, stop=False)
                    nc.tensor.matmul(ps_rb[:], selb[:], s1[0:1, :],
                                     start=False, stop=True)
                    rec_sb = recp.tile([P, R], F32, tag="rec", name=f"rec{p}")
                    nc.vector.reciprocal_approx_fast(out=rec_sb[:], in_=ps_rb[:])
                    nc.vector.tensor_tensor(ot[p][:], ot[p][:], rec_sb[:], op=ALU.mult)

                for p in range(H // 2):
                    emit_k(p)
                    if p < 2:
                        emit_v1(4 * p)
                        emit_v1(4 * p + 1)
                        emit_v1(4 * p + 2)
                        emit_v1(4 * p + 3)
                    if p > 0:
                        emit_o(p - 1)
                        emit_norm(p - 1)
                    emit_qk(p)
                emit_o(H // 2 - 1)
                emit_norm(H // 2 - 1)

        # ================= attn out projection + residual =================
        with tc.tile_pool(name="xres", bufs=3) as xresp, \
             tc.tile_pool(name="mm_ps2", bufs=3, space="PSUM") as mm_ps:
            wo_sb = [wrow.tile([P, C], BF16, tag="w", name=f"wo{i}") for i in range(KC)]
            for k in range(KC):
                nc.sync.dma_start(out=wo_sb[k][:], in_=io["wo"][k * P:(k + 1) * P, :])
            for m in range(KC):
                ps = mm_ps.tile([P, 512], F32, tag="mm")
                for k in range(KC):
                    nc.tensor.matmul(ps[:], wo_sb[k][:, m * P:(m + 1) * P], ot[k][:],
                                     start=(k == 0), stop=(k == KC - 1))
                t1 = tmpp.tile([P, 512], F32, tag="tmp")
                nc.scalar.activation(t1[:], ps[:], AF.Identity, bias=bo_sb[:, m:m + 1])
                xr = xresp.tile([P, R], F32R, tag="xr")
                nc.sync.dma_start(out=xr[:], in_=io["xT"][m * P:(m + 1) * P, 0:R])
                nc.vector.tensor_add(x2[m][:], t1[:], xr[:].bitcast(F32))

        # ================= LN2 =================
        with tc.tile_pool(name="ln_ps2", bufs=2, space="PSUM") as ln_ps, \
             tc.tile_pool(name="ln_bc2", bufs=2, space="PSUM") as ln_bc:

            def get_x2(k, chunk, use):
                if use == 0:
                    xcr = sqp.tile([P, 512], F32R, tag="xcr", name=f"x2r{k}", bufs=2)
                    nc.vector.tensor_copy(out=xcr[:], in_=x2[k][:])
                    return xcr[:], x2[k][:]
                return None, x2[k][:]

            def put_xn2(k, chunk, t1, ps_rstd):
                nc.vector.tensor_tensor(xn2[k][:], t1, ps_rstd, op=ALU.mult)

            _ln_fm(nc, ln_ps, ln_bc, vecp, sqp, tmpp, ones_kr, ones_c1, eps_sb,
                   get_x2, put_xn2, 1, "ln2")

        # ================= MLP =================
        # fc1 + gelu: process w1 in 4 column groups of 1024
        h1p = ctx.enter_context(tc.tile_pool(name="h1p", bufs=DFF // P))
        h1 = [h1p.tile([P, R], BF16, tag="h1", name=f"h1_{i}") for i in range(DFF // P)]
        with tc.tile_pool(name="mm_ps3", bufs=3, space="PSUM") as mm_ps:
            for cg in range(4):
                w1_sb = [wrow.tile([P, C], BF16, tag="w", name=f"w1_{cg}_{i}")
                         for i in range(KC)]
                for k in range(KC):
                    nc.sync.dma_start(out=w1_sb[k][:],
                                      in_=io["w1"][k * P:(k + 1) * P, cg * C:(cg + 1) * C])
                for m in range(KC):
                    ps = mm_ps.tile([P, 512], F32, tag="mm")
                    for k in range(KC):
                        nc.tensor.matmul(ps[:], w1_sb[k][:, m * P:(m + 1) * P], xn2[k][:],
                                         start=(k == 0), stop=(k == KC - 1))
                    om = cg * KC + m
                    nc.scalar.activation(h1[om][:], ps[:], AF.Gelu_apprx_tanh,
                                         bias=b1_sb[:, om:om + 1])

        # fc2 + residual: two output column groups of 512 (4 psum banks each)
        with tc.tile_pool(name="fc2ps", bufs=4, space="PSUM") as fc2ps:
            for mg in range(2):
                ps_list = [fc2ps.tile([P, 512], F32, tag="fc2", name=f"fc2ps{mg}_{i}")
                           for i in range(4)]
                w2_sb = [wrow.tile([P, 512], BF16, tag="w2s", name=f"w2_{mg}_{i}", bufs=8)
                         for i in range(DFF // P)]
                for k in range(DFF // P):
                    nc.sync.dma_start(out=w2_sb[k][:],
                                      in_=io["w2"][k * P:(k + 1) * P, mg * 512:(mg + 1) * 512])
                for k in range(DFF // P):
                    for m in range(4):
                        nc.tensor.matmul(ps_list[m][:], w2_sb[k][:, m * P:(m + 1) * P],
                                         h1[k][:], start=(k == 0), stop=(k == DFF // P - 1))
                for m in range(4):
                    om = mg * 4 + m
                    t2 = tmpp.tile([P, 512], F32, tag="tmp", name=f"t2_{om}")
                    nc.scalar.activation(t2[:], ps_list[m][:], AF.Identity,
                                         bias=b2_sb[:, om:om + 1])
                    y_sb = yp.tile([P, R], F32, tag="y", name=f"y{om}")
                    nc.vector.tensor_add(y_sb[:], t2[:], x2[om][:])
                    nc.sync.dma_start(out=io["yT"][om * P:(om + 1) * P, :], in_=y_sb[:])

    if not nc.is_finalized():
        nc.finalize()
    return nc


def _get_nc():
    if "nc" not in _CACHE:
        _CACHE["nc"] = _build()
    return _CACHE["nc"]


def _prep_in_maps(inputs):
    x = np.asarray(inputs["x"], dtype=np.float32)
    mask = np.asarray(inputs["mask"])
    scale = float(D) ** -0.5
    wq = (np.asarray(inputs["wq"], np.float32) * scale).astype(BFNP)
    bq = (np.asarray(inputs["bq"], np.float32) * scale).astype(np.float32)
    wkv = np.asarray(inputs["wkv"], np.float32)
    bkv = np.asarray(inputs["bkv"], np.float32)
    wk = np.ascontiguousarray(wkv[:, :C]).astype(BFNP)
    wv = np.ascontiguousarray(wkv[:, C:]).astype(BFNP)
    bk = np.ascontiguousarray(bkv[:C]).astype(np.float32)
    bv = np.ascontiguousarray(bkv[C:]).astype(np.float32)
    wo = np.asarray(inputs["wo"], np.float32).astype(BFNP)
    bo = np.asarray(inputs["bo"], np.float32)
    w1 = np.asarray(inputs["w1"], np.float32).astype(BFNP)
    b1 = np.asarray(inputs["b1"], np.float32)
    w2 = np.asarray(inputs["w2"], np.float32).astype(BFNP)
    b2 = np.asarray(inputs["b2"], np.float32)
    mask01 = mask.astype(np.float32)

    sel2 = np.zeros((2, P), dtype=np.float32)
    sel2[0, 0:D] = 1.0
    sel2[1, D:2 * D] = 1.0
    shared = dict(wq=wq, wk=wk, wv=wv, wo=wo, w1=w1, w2=w2,
                  bq=bq, bk=bk, bv=bv, bo=bo, b1=b1, b2=b2, sel2=sel2)
    in_maps = []
    for c in range(8):
        b = c // 2
        rh = c % 2
        own = np.arange(rh * R, rh * R + R)
        oth = np.arange((1 - rh) * R, (1 - rh) * R + R)
        perm = np.concatenate([own, oth])
        xT = np.ascontiguousarray(x[b].T[:, perm])
        mskT = np.ascontiguousarray(mask01[np.ix_(own, perm)].T).astype(BFNP)
        m = dict(shared)
        m["xT"] = xT
        m["mskT"] = mskT
        in_maps.append(m)
    return in_maps


def _assemble(results):
    out = np.empty((B, N, C), dtype=np.float32)
    for c in range(8):
        b = c // 2
        rh = c % 2
        out[b, rh * R:(rh + 1) * R, :] = results[c]["yT"].T
    return out


def run(inputs, trace=False):
    nc = _get_nc()
    in_maps = _prep_in_maps(inputs)
    res = run_bass_kernel_spmd(nc, in_maps, core_ids=list(range(8)), trace=trace)
    return _assemble(res.results), res


def kernel(**inputs):
    out, _ = run(inputs, trace=False)
    return out


# revision 25
# speedup vs baseline: 1.3081x; 1.0227x over previous
"""Trainium2 Bass kernel for a pre-norm transformer block (attention + MLP).

Sharding: pure data-parallel over 8 cores. Core c handles batch b=c//2 and
query-row half rh=c%2 (512 tokens). K/V are computed for the full 1024-token
batch on every core (duplicated across the pair) so no collectives are needed.

Device layout: activations are feature-major (features on partitions, tokens
on the free dim) so matmul chains need no transposes. Host permutes tokens so
each core's own 512 query tokens are always columns 0:512. LayerNorm is done
feature-major with ones-matmul partition reductions (float32r full-rate
matmuls); softmax uses exp(sim) * mask01 (no max subtraction, exact because
masked logits contribute exp=0) with per-query sums obtained for free from a
ones-column appended to V.
"""

import numpy as np
import ml_dtypes
from contextlib import ExitStack

import concourse.bass as bass
from concourse.bacc import Bacc
import concourse.tile as tile
from concourse import mybir
from concourse.bass_utils import run_bass_kernel_spmd

F32 = mybir.dt.float32
F32R = mybir.dt.float32r
BF16 = mybir.dt.bfloat16
AF = mybir.ActivationFunctionType
ALU = mybir.AluOpType
BFNP = ml_dtypes.bfloat16

B, N, C = 4, 1024, 1024
H, D = 16, 64
DFF = 4096
R = 512          # own query rows per core
P = 128
KC = C // P      # 8 feature k-tiles
NT = N // P      # 8 token tiles
EPS = 1e-6

_CACHE: dict = {}


def _r32(ap):
    return ap.bitcast(F32R)


def _ln_fm(nc, ln_ps, ln_bc, vecp, sqp, tmpp, ones_kr, ones_c1, eps_sb,
           get_x, put_xn, n_chunks, tagpfx):
    """Streaming feature-major layernorm over KC partition tiles.

    get_x(k, chunk, use) -> sbuf fp32 AP [128, 512] for feature tile k,
    token chunk `chunk` (use is 0 for the stats pass, 1 for normalize).
    put_xn(k, chunk, t1_ap, ps_rstd_ap) stores (x-mu)*rstd.
    """
    inv_c = 1.0 / C
    for chunk in range(n_chunks):
        ps_s = ln_ps.tile([1, 512], F32, tag="lnstat", name=f"{tagpfx}s{chunk}")
        ps_q = ln_ps.tile([1, 512], F32, tag="lnstat", name=f"{tagpfx}q{chunk}")
        for k in range(KC):
            xr, xf = get_x(k, chunk, 0)
            sqc = sqp.tile([P, 512], F32R, tag="sq", name=f"{tagpfx}sq{chunk}_{k}", bufs=2)
            nc.vector.tensor_mul(sqc[:], xf, xf)
            nc.tensor.matmul(ps_s[:], ones_kr[:], xr,
                             start=(k == 0), stop=(k == KC - 1))
            nc.tensor.matmul(ps_q[:], ones_kr[:], sqc[:],
                             start=(k == 0), stop=(k == KC - 1))
        mu = vecp.tile([1, 512], F32, tag="vec", name=f"{tagpfx}mu{chunk}")
        var = vecp.tile([1, 512], F32, tag="vec", name=f"{tagpfx}var{chunk}")
        rstd = vecp.tile([1, 512], F32, tag="vec", name=f"{tagpfx}rstd{chunk}")
        nc.scalar.mul(mu[:], ps_s[:], inv_c)
        nc.scalar.mul(var[:], ps_q[:], inv_c)          # E[x^2]
        msq = vecp.tile([1, 512], F32, tag="vec", name=f"{tagpfx}msq{chunk}")
        nc.vector.tensor_mul(msq[:], mu[:], mu[:])
        nc.vector.tensor_sub(var[:], var[:], msq[:])
        nc.scalar.activation(var[:], var[:], AF.Sqrt, bias=eps_sb[:])
        nc.vector.reciprocal_approx_fast(out=rstd[:], in_=var[:])
        mu_r = vecp.tile([1, 512], F32R, tag="vecr", name=f"{tagpfx}mur{chunk}", bufs=2)
        rstd_r = vecp.tile([1, 512], F32R, tag="vecr", name=f"{tagpfx}rsr{chunk}", bufs=2)
        nc.scalar.copy(mu_r[:], mu[:])
        nc.scalar.copy(rstd_r[:], rstd[:])
        ps_mu = ln_bc.tile([P, 512], F32, tag="lnbc", name=f"{tagpfx}bmu{chunk}")
        ps_rstd = ln_bc.tile([P, 512], F32, tag="lnbc", name=f"{tagpfx}brs{chunk}")
        nc.tensor.matmul(ps_mu[:], ones_c1[:, 0:P], mu_r[:],
                         start=True, stop=True)
        nc.tensor.matmul(ps_rstd[:], ones_c1[:, 0:P], rstd_r[:],
                         start=True, stop=True)
        for k in range(KC):
            _, xf = get_x(k, chunk, 1)
            t1 = tmpp.tile([P, 512], F32, tag="tmp", name=f"{tagpfx}t{chunk}_{k}")
            nc.vector.tensor_sub(t1[:], xf, ps_mu[:])
            put_xn(k, chunk, t1[:], ps_rstd[:])


def _build():
    nc = Bacc()
    io = {}
    io["xT"] = nc.dram_tensor("xT", [C, N], F32R, kind="ExternalInput")
    io["mskT"] = nc.dram_tensor("mskT", [N, R], BF16, kind="ExternalInput")
    for nm, shp in [("wq", [C, C]), ("wk", [C, C]), ("wv", [C, C]),
                    ("wo", [C, C]), ("w1", [C, DFF]), ("w2", [DFF, C])]:
        io[nm] = nc.dram_tensor(nm, shp, BF16, kind="ExternalInput")
    for nm, n_ in [("bq", C), ("bk", C), ("bv", C), ("bo", C), ("b1", DFF), ("b2", C)]:
        io[nm] = nc.dram_tensor(nm, [n_], F32, kind="ExternalInput")
    io["sel2"] = nc.dram_tensor("sel2", [2, P], F32R, kind="ExternalInput")
    io["yT"] = nc.dram_tensor("yT", [C, R], F32, kind="ExternalOutput")

    def bias_cols(name, n_):
        # bias vector [n_] -> sbuf [128, n_//128], col m = b[m*128:(m+1)*128]
        return bass.AP(tensor=io[name][:].tensor, offset=0, ap=[[1, P], [P, n_ // P]])

    with tile.TileContext(nc) as tc, ExitStack() as ctx:
        # ---- long-lived sbuf pools (stack: first opened = last closed)
        const = ctx.enter_context(tc.tile_pool(name="const", bufs=1))
        x2p = ctx.enter_context(tc.tile_pool(name="x2p", bufs=KC))
        xn2p = ctx.enter_context(tc.tile_pool(name="xn2p", bufs=KC))
        yp = ctx.enter_context(tc.tile_pool(name="yp", bufs=2))
        mskp = ctx.enter_context(tc.tile_pool(name="mskp", bufs=NT))
        qtp = ctx.enter_context(tc.tile_pool(name="qtp", bufs=KC))
        ktp = ctx.enter_context(tc.tile_pool(name="ktp", bufs=KC))
        vtp = ctx.enter_context(tc.tile_pool(name="vtp", bufs=NT))
        otp = ctx.enter_context(tc.tile_pool(name="otp", bufs=KC))
        wrow = ctx.enter_context(tc.tile_pool(name="wrow", bufs=10))
        vecp = ctx.enter_context(tc.tile_pool(name="vecp", bufs=4))
        tmpp = ctx.enter_context(tc.tile_pool(name="tmpp", bufs=3))
        sqp = ctx.enter_context(tc.tile_pool(name="sqp", bufs=3))

        # ---- constants
        bq_sb = const.tile([P, C // P], F32)
        bk_sb = const.tile([P, C // P], F32)
        bo_sb = const.tile([P, C // P], F32)
        b1_sb = const.tile([P, DFF // P], F32)
        b2_sb = const.tile([P, C // P], F32)
        nc.sync.dma_start(out=bq_sb[:], in_=bias_cols("bq", C))
        nc.sync.dma_start(out=bk_sb[:], in_=bias_cols("bk", C))
        nc.sync.dma_start(out=bo_sb[:], in_=bias_cols("bo", C))
        nc.sync.dma_start(out=b1_sb[:], in_=bias_cols("b1", DFF))
        nc.sync.dma_start(out=b2_sb[:], in_=bias_cols("b2", C))
        bv_b = const.tile([P, C], F32)
        nc.sync.dma_start(out=bv_b[:], in_=bass.AP(tensor=io["bv"][:].tensor,
                                                   offset=0, ap=[[0, P], [1, C]]))
        ones_kf = const.tile([P, 1], F32)
        nc.vector.memset(ones_kf[:], 1.0)
        ones_kr = const.tile([P, 1], F32R)
        nc.vector.tensor_copy(out=ones_kr[:], in_=ones_kf[:])
        ones_cf = const.tile([1, P], F32)
        nc.vector.memset(ones_cf[:], 1.0)
        ones_c1 = const.tile([1, P], F32R)
        nc.vector.tensor_copy(out=ones_c1[:], in_=ones_cf[:])
        eps_sb = const.tile([1, 1], F32)
        nc.vector.memset(eps_sb[:], EPS)
        sela = const.tile([1, P], F32R)
        selb = const.tile([1, P], F32R)
        nc.sync.dma_start(out=sela[:], in_=io["sel2"][0:1, :])
        nc.sync.dma_start(out=selb[:], in_=io["sel2"][1:2, :])

        msk_sb = [mskp.tile([P, R], BF16, tag="msk", name=f"msk{i}") for i in range(NT)]
        for t in range(NT):
            nc.sync.dma_start(out=msk_sb[t][:], in_=io["mskT"][t * P:(t + 1) * P, :])

        # resident activation tiles
        qt = [qtp.tile([P, R], BF16, tag="qt", name=f"qt{i}") for i in range(KC)]
        kt = [ktp.tile([P, N], BF16, tag="kt", name=f"kt{i}") for i in range(KC)]
        vt = [vtp.tile([P, H * (D + 1)], BF16, tag="vt", name=f"vt{i}") for i in range(NT)]
        ot = [otp.tile([P, R], BF16, tag="ot", name=f"ot{i}") for i in range(KC)]
        x2 = [x2p.tile([P, R], F32, tag="x2", name=f"x2_{i}") for i in range(KC)]
        xn2 = [xn2p.tile([P, R], BF16, tag="xn2", name=f"xn2_{i}") for i in range(KC)]

        # ================= LN1 / Q / V0 then interleaved K+attention =================
        with tc.tile_pool(name="xn1p", bufs=KC) as xn1p:
            xn1 = [xn1p.tile([P, N], BF16, tag="xn1", name=f"xn1_{i}") for i in range(KC)]

            with tc.tile_pool(name="xs1", bufs=11) as xs1, \
                 tc.tile_pool(name="ln_ps", bufs=2, space="PSUM") as ln_ps, \
                 tc.tile_pool(name="ln_bc", bufs=2, space="PSUM") as ln_bc, \
                 tc.tile_pool(name="mm_ps", bufs=3, space="PSUM") as mm_ps:

                # HAM warmup: dummy back-to-back matmuls while LN1 does DMA/DVE work
                wup = mm_ps.tile([P, 512], F32, tag="wup", bufs=1)
                for i in range(24):
                    nc.tensor.matmul(wup[:], msk_sb[0][:, 0:P], msk_sb[0][:],
                                     start=(i == 0), stop=(i == 23))
                wup_sb = vecp.tile([1, 1], F32, tag="vec", name="wupsb")
                nc.scalar.copy(wup_sb[:], wup[0:1, 0:1])

                x1cache = {}

                def get_x1(k, chunk, use):
                    if (k, chunk) not in x1cache:
                        xc = xs1.tile([P, 512], F32R, tag="xs", name=f"xa_{chunk}_{k}")
                        eng = nc.gpsimd if k % 2 == 0 else nc.sync
                        eng.dma_start(out=xc[:], in_=io["xT"][k * P:(k + 1) * P,
                                                              chunk * 512:(chunk + 1) * 512])
                        x1cache[(k, chunk)] = xc
                    xc = x1cache[(k, chunk)]
                    return xc[:], xc[:].bitcast(F32)

                def put_xn1(k, chunk, t1, ps_rstd):
                    nc.vector.tensor_tensor(xn1[k][:, chunk * 512:(chunk + 1) * 512],
                                            t1, ps_rstd, op=ALU.mult)

                _ln_fm(nc, ln_ps, ln_bc, vecp, sqp, tmpp, ones_kr, ones_c1, eps_sb,
                       get_x1, put_xn1, 2, "ln1")

                # ---- Q projection (own tokens only)
                wq_sb = [wrow.tile([P, C], BF16, tag="w", name=f"wq{i}") for i in range(KC)]
                for k in range(KC):
                    nc.sync.dma_start(out=wq_sb[k][:], in_=io["wq"][k * P:(k + 1) * P, :])
                for m in range(KC):
                    ps = mm_ps.tile([P, 512], F32, tag="mm")
                    for k in range(KC):
                        nc.tensor.matmul(ps[:], wq_sb[k][:, m * P:(m + 1) * P], xn1[k][:, 0:R],
                                         start=(k == 0), stop=(k == KC - 1))
                    nc.scalar.activation(qt[m][:], ps[:], AF.Identity, bias=bq_sb[:, m:m + 1])

                # ---- V projection chunk 0 (heads 0-7), token-major with ones column
                wv_sb = [wrow.tile([P, C], BF16, tag="w", name=f"wv{i}") for i in range(KC)]
                for k in range(KC):
                    nc.sync.dma_start(out=wv_sb[k][:], in_=io["wv"][k * P:(k + 1) * P, :])
                for t in range(NT):
                    vre = vt[t][:].rearrange("p (h j) -> p h j", j=D + 1)
                    nc.vector.memset(vre[:, :, D:D + 1], 1.0)
                    ps = mm_ps.tile([P, 512], F32, tag="mm")
                    for k in range(KC):
                        nc.tensor.matmul(ps[:], xn1[k][:, t * P:(t + 1) * P],
                                         wv_sb[k][:, 0:512],
                                         start=(k == 0), stop=(k == KC - 1))
                    nc.vector.tensor_tensor(
                        vre[:, 0:8, 0:D], ps[:].rearrange("p (h j) -> p h j", j=D),
                        bv_b[:, 0:512].rearrange("p (h j) -> p h j", j=D),
                        op=ALU.add)

            # ---- interleaved: K projection + V chunk 1 + attention pair pipeline
            with tc.tile_pool(name="simps", bufs=2, space="PSUM") as simps, \
                 tc.tile_pool(name="ops", bufs=1, space="PSUM") as ops_, \
                 tc.tile_pool(name="mmb", bufs=2, space="PSUM") as mmb, \
                 tc.tile_pool(name="rbps", bufs=1, space="PSUM") as rbps, \
                 tc.tile_pool(name="a2p", bufs=10) as a2p, \
                 tc.tile_pool(name="recp", bufs=2) as recp, \
                 tc.tile_pool(name="smr", bufs=2) as smr:

                wk_sb = [wrow.tile([P, C], BF16, tag="w", name=f"wk{i}") for i in range(KC)]
                for k in range(KC):
                    nc.sync.dma_start(out=wk_sb[k][:], in_=io["wk"][k * P:(k + 1) * P, :])
                wvh_sb = [wrow.tile([P, 512], BF16, tag="w2s", name=f"wvh{i}", bufs=8)
                          for i in range(KC)]
                for k in range(KC):
                    nc.sync.dma_start(out=wvh_sb[k][:],
                                      in_=io["wv"][k * P:(k + 1) * P, 512:1024])

                a_tiles = {}
                sums = {}

                def emit_k(p):
                    for nn_ in range(2):
                        ps = mmb.tile([P, 512], F32, tag="mm", name=f"kp{p}_{nn_}")
                        for k in range(KC):
                            nc.tensor.matmul(ps[:], wk_sb[k][:, p * P:(p + 1) * P],
                                             xn1[k][:, nn_ * 512:(nn_ + 1) * 512],
                                             start=(k == 0), stop=(k == KC - 1))
                        nc.scalar.activation(kt[p][:, nn_ * 512:(nn_ + 1) * 512], ps[:],
                                             AF.Identity, bias=bk_sb[:, p:p + 1])

                def emit_v1(t):
                    vre = vt[t][:].rearrange("p (h j) -> p h j", j=D + 1)
                    ps = mmb.tile([P, 512], F32, tag="mm", name=f"v1_{t}")
                    for k in range(KC):
                        nc.tensor.matmul(ps[:], xn1[k][:, t * P:(t + 1) * P],
                                         wvh_sb[k][:],
                                         start=(k == 0), stop=(k == KC - 1))
                    nc.vector.tensor_tensor(
                        vre[:, 8:16, 0:D], ps[:].rearrange("p (h j) -> p h j", j=D),
                        bv_b[:, 512:1024].rearrange("p (h j) -> p h j", j=D),
                        op=ALU.add)

                def emit_qk(p):
                    kth0 = kt[p][0:D, :]
                    kth1 = kt[p][D:2 * D, :]
                    qth0 = qt[p][0:D, :]
                    qth1 = qt[p][D:2 * D, :]
                    for tk in range(NT):
                        ps2 = simps.tile([P, 2 * R], F32, tag="sim", name=f"sim{p}_{tk}")
                        nc.tensor.matmul(ps2[:, 0:R], kth0[:, tk * P:(tk + 1) * P], qth0[:],
                                         start=True, stop=True)
                        nc.tensor.matmul(ps2[:, R:2 * R], kth1[:, tk * P:(tk + 1) * P], qth1[:],
                                         start=True, stop=True)
                        a2 = a2p.tile([P, 2 * R], BF16, tag="a", name=f"a{p}_{tk}")
                        nc.scalar.activation(a2[:], ps2[:], AF.Exp)
                        nc.vector.tensor_mul(a2[:, 0:R], a2[:, 0:R], msk_sb[tk][:])
                        nc.vector.tensor_mul(a2[:, R:2 * R], a2[:, R:2 * R], msk_sb[tk][:])
                        a_tiles[(p, tk)] = a2

                def emit_o(p):
                    s0 = smr.tile([1, R], F32R, tag="s0", name=f"s0_{p}")
                    s1 = smr.tile([1, R], F32R, tag="s1", name=f"s1_{p}")
                    sums[p] = (s0, s1)
                    for hh in range(2):
                        h = 2 * p + hh
                        ps_o = ops_.tile([D + 1, R], F32, tag="o", name=f"o{h}")
                        for tk in range(NT):
                            vre = vt[tk][:].rearrange("p (h j) -> p h j", j=D + 1)
                            nc.tensor.matmul(ps_o[:], vre[:, h, 0:D + 1],
                                             a_tiles[(p, tk)][:, hh * R:(hh + 1) * R],
                                             start=(tk == 0), stop=(tk == NT - 1))
                        dst = s0 if hh == 0 else s1
                        nc.scalar.copy(dst[0:1, :], ps_o[D:D + 1, :])
                        nc.scalar.copy(ot[p][hh * D:(hh + 1) * D, :], ps_o[0:D, :])
                    for tk in range(NT):
                        del a_tiles[(p, tk)]

                def emit_norm(p):
                    s0, s1 = sums.pop(p)
                    ps_rb = rbps.tile([P, R], F32, tag="rb", name=f"rb{p}")
                    nc.tensor.matmul(ps_rb[:], sela[:], s0[0:1, :],
                                     start=True, stop=False)
                    nc.tensor.matmul(ps_rb[:], selb[:], s1[0:1, :],
                                     start=False, stop=True)
                    rec_sb = recp.tile([P, R], F32, tag="rec", name=f"rec{p}")
                    nc.vector.reciprocal_approx_fast(out=rec_sb[:], in_=ps_rb[:])
                    nc.vector.tensor_tensor(ot[p][:], ot[p][:], rec_sb[:], op=ALU.mult)

                for p in range(H // 2):
                    emit_k(p)
                    if p < 2:
                        emit_v1(4 * p)
                        emit_v1(4 * p + 1)
                        emit_v1(4 * p + 2)
                        emit_v1(4 * p + 3)
                    if p > 0:
                        emit_o(p - 1)
                        emit_norm(p - 1)
                    emit_qk(p)
                emit_o(H // 2 - 1)
                emit_norm(H // 2 - 1)

        # ================= attn out projection + residual =================
        with tc.tile_pool(name="xres", bufs=3) as xresp, \
             tc.tile_pool(name="mm_ps2", bufs=3, space="PSUM") as mm_ps:
            wo_sb = [wrow.tile([P, C], BF16, tag="w", name=f"wo{i}") for i in range(KC)]
            for k in range(KC):
                nc.sync.dma_start(out=wo_sb[k][:], in_=io["wo"][k * P:(k + 1) * P, :])
            for m in range(KC):
                ps = mm_ps.tile([P, 512], F32, tag="mm")
                for k in range(KC):
                    nc.tensor.matmul(ps[:], wo_sb[k][:, m * P:(m + 1) * P], ot[k][:],
                                     start=(k == 0), stop=(k == KC - 1))
                xr = xresp.tile([P, R], F32R, tag="xr")
                nc.sync.dma_start(out=xr[:], in_=io["xT"][m * P:(m + 1) * P, 0:R])
                nc.vector.scalar_tensor_tensor(x2[m][:], ps[:], bo_sb[:, m:m + 1],
                                               xr[:].bitcast(F32),
                                               op0=ALU.add, op1=ALU.add)

        # ================= LN2 =================
        with tc.tile_pool(name="ln_ps2", bufs=2, space="PSUM") as ln_ps, \
             tc.tile_pool(name="ln_bc2", bufs=2, space="PSUM") as ln_bc:

            def get_x2(k, chunk, use):
                if use == 0:
                    xcr = sqp.tile([P, 512], F32R, tag="xcr", name=f"x2r{k}", bufs=2)
                    nc.vector.tensor_copy(out=xcr[:], in_=x2[k][:])
                    return xcr[:], x2[k][:]
                return None, x2[k][:]

            def put_xn2(k, chunk, t1, ps_rstd):
                nc.vector.tensor_tensor(xn2[k][:], t1, ps_rstd, op=ALU.mult)

            _ln_fm(nc, ln_ps, ln_bc, vecp, sqp, tmpp, ones_kr, ones_c1, eps_sb,
                   get_x2, put_xn2, 1, "ln2")

        # ================= MLP =================
        # fc1 + gelu: process w1 in 4 column groups of 1024
        h1p = ctx.enter_context(tc.tile_pool(name="h1p", bufs=DFF // P))
        h1 = [h1p.tile([P, R], BF16, tag="h1", name=f"h1_{i}") for i in range(DFF // P)]
        with tc.tile_pool(name="mm_ps3", bufs=3, space="PSUM") as mm_ps:
            for cg in range(4):
                w1_sb = [wrow.tile([P, C], BF16, tag="w", name=f"w1_{cg}_{i}")
                         for i in range(KC)]
                for k in range(KC):
                    nc.sync.dma_start(out=w1_sb[k][:],
                                      in_=io["w1"][k * P:(k + 1) * P, cg * C:(cg + 1) * C])
                for m in range(KC):
                    ps = mm_ps.tile([P, 512], F32, tag="mm")
                    for k in range(KC):
                        nc.tensor.matmul(ps[:], w1_sb[k][:, m * P:(m + 1) * P], xn2[k][:],
                                         start=(k == 0), stop=(k == KC - 1))
                    om = cg * KC + m
                    nc.scalar.activation(h1[om][:], ps[:], AF.Gelu_apprx_tanh,
                                         bias=b1_sb[:, om:om + 1])

        # fc2 + residual: two output column groups of 512 (4 psum banks each)
        with tc.tile_pool(name="fc2ps", bufs=4, space="PSUM") as fc2ps:
            for mg in range(2):
                ps_list = [fc2ps.tile([P, 512], F32, tag="fc2", name=f"fc2ps{mg}_{i}")
                           for i in range(4)]
                w2_sb = [wrow.tile([P, 512], BF16, tag="w2s", name=f"w2_{mg}_{i}", bufs=8)
                         for i in range(DFF // P)]
                for k in range(DFF // P):
                    nc.sync.dma_start(out=w2_sb[k][:],
                                      in_=io["w2"][k * P:(k + 1) * P, mg * 512:(mg + 1) * 512])
                for k in range(DFF // P):
                    for m in range(4):
                        nc.tensor.matmul(ps_list[m][:], w2_sb[k][:, m * P:(m + 1) * P],
                                         h1[k][:], start=(k == 0), stop=(k == DFF // P - 1))
                for m in range(4):
                    om = mg * 4 + m
                    y_sb = yp.tile([P, R], F32, tag="y", name=f"y{om}")
                    nc.vector.scalar_tensor_tensor(y_sb[:], ps_list[m][:],
                                                   b2_sb[:, om:om + 1], x2[om][:],
                                                   op0=ALU.add, op1=ALU.add)
                    nc.sync.dma_start(out=io["yT"][om * P:(om + 1) * P, :], in_=y_sb[:])

    if not nc.is_finalized():
        nc.finalize()
    return nc


def _get_nc():
    if "nc" not in _CACHE:
        _CACHE["nc"] = _build()
    return _CACHE["nc"]


def _prep_in_maps(inputs):
    x = np.asarray(inputs["x"], dtype=np.float32)
    mask = np.asarray(inputs["mask"])
    scale = float(D) ** -0.5
    wq = (np.asarray(inputs["wq"], np.float32) * scale).astype(BFNP)
    bq = (np.asarray(inputs["bq"], np.float32) * scale).astype(np.float32)
    wkv = np.asarray(inputs["wkv"], np.float32)
    bkv = np.asarray(inputs["bkv"], np.float32)
    wk = np.ascontiguousarray(wkv[:, :C]).astype(BFNP)
    wv = np.ascontiguousarray(wkv[:, C:]).astype(BFNP)
    bk = np.ascontiguousarray(bkv[:C]).astype(np.float32)
    bv = np.ascontiguousarray(bkv[C:]).astype(np.float32)
    wo = np.asarray(inputs["wo"], np.float32).astype(BFNP)
    bo = np.asarray(inputs["bo"], np.float32)
    w1 = np.asarray(inputs["w1"], np.float32).astype(BFNP)
    b1 = np.asarray(inputs["b1"], np.float32)
    w2 = np.asarray(inputs["w2"], np.float32).astype(BFNP)
    b2 = np.asarray(inputs["b2"], np.float32)
    mask01 = mask.astype(np.float32)

    sel2 = np.zeros((2, P), dtype=np.float32)
    sel2[0, 0:D] = 1.0
    sel2[1, D:2 * D] = 1.0
    shared = dict(wq=wq, wk=wk, wv=wv, wo=wo, w1=w1, w2=w2,
                  bq=bq, bk=bk, bv=bv, bo=bo, b1=b1, b2=b2, sel2=sel2)
    in_maps = []
    for c in range(8):
        b = c // 2
        rh = c % 2
        own = np.arange(rh * R, rh * R + R)
        oth = np.arange((1 - rh) * R, (1 - rh) * R + R)
        perm = np.concatenate([own, oth])
        xT = np.ascontiguousarray(x[b].T[:, perm])
        mskT = np.ascontiguousarray(mask01[np.ix_(own, perm)].T).astype(BFNP)
        m = dict(shared)
        m["xT"] = xT
        m["mskT"] = mskT
        in_maps.append(m)
    return in_maps


def _assemble(results):
    out = np.empty((B, N, C), dtype=np.float32)
    for c in range(8):
        b = c // 2
        rh = c % 2
        out[b, rh * R:(rh + 1) * R, :] = results[c]["yT"].T
    return out


def run(inputs, trace=False):
    nc = _get_nc()
    in_maps = _prep_in_maps(inputs)
    res = run_bass_kernel_spmd(nc, in_maps, core_ids=list(range(8)), trace=trace)
    return _assemble(res.results), res


def kernel(**inputs):
    out, _ = run(inputs, trace=False)
    return out
